# revision 1
# baseline (speedup 1.0000x reference)
"""Trainium2 Bass kernel for nn_MultiHeadAttention_66322884984909.

Math (faithful to reference):
  Q = X @ W_Q.T reshaped (B, H, L, hd) via DIRECT reshape -> head h owns rows
  128h:128(h+1) of the projected (L, D) matrix, reinterpreted as (L=2048, hd=64).
  Heads are therefore sequence-parallel: the whole computation decomposes over
  the 32 (batch, head) pairs with no cross-pair coupling. 8 cores x 4 pairs.

Per pair (X_s = X[b, 128h:128h+128, :], shape (128, 1024)):
  Qf = X_s @ W_Q.T        (128, 1024)  -> Qh = Qf.reshape(2048, 64)
  S  = Qh @ Kh.T          (2048, 2048) causal-masked softmax (no scaling)
  O  = softmax(S) @ Vh    (2048, 64)
  Y  = O.reshape(128, 1024) @ W_O.T + b_O   -> out rows 128h:128(h+1) of batch b

No max-subtraction in softmax: logits ~ N(0, 64), |S| < 80 with overwhelming
probability, exp stays finite in fp32. Row sums come free as a 65th ones-column
appended to V in the P@V matmul. All matmuls run in fp32r (full-rate tensor
engine mode, ~1e-4 relative error).
"""

import numpy as np

import concourse.bass as bass
from concourse import bacc
import concourse.mybir as mybir
import concourse.tile as tile
from concourse.bass_utils import run_bass_kernel_spmd
from concourse.masks import make_identity

F32 = mybir.dt.float32
F32R = mybir.dt.float32r
EXP = mybir.ActivationFunctionType.Exp

B, L, D = 2, 2048, 1024
H, HD = 16, 64
NCORES = 8
PPC = 4  # pairs per core


def build_nc(repeat=1):
    nc = bacc.Bacc(trn_type="TRN2", target_bir_lowering=False, debug=False)

    xt = nc.declare_dram_parameter("xt", [PPC, 1024, 128], F32R, isOutput=False)
    wq = nc.declare_dram_parameter("wq", [1024, 1024], F32R, isOutput=False)   # W_Q.T
    wk = nc.declare_dram_parameter("wk", [1024, 1024], F32R, isOutput=False)   # W_K.T
    wv = nc.declare_dram_parameter("wv", [1024, 1024], F32R, isOutput=False)   # W_V.T
    wo = nc.declare_dram_parameter("wo", [64, 16 * 1024], F32R, isOutput=False)
    bias = nc.declare_dram_parameter("bias", [128, 1024], F32, isOutput=False)
    ones = nc.declare_dram_parameter("ones", [128, 16], F32R, isOutput=False)
    out = nc.declare_dram_parameter("out", [PPC, 128, 1024], F32, isOutput=True)
    qsh = nc.dram_tensor("qsh", [PPC // 2, 128, 2048], F32R)
    ksh = nc.dram_tensor("ksh", [PPC // 2, 128, 2048], F32R)
    vsh = nc.dram_tensor("vsh", [PPC, 128, 1040], F32R)

    with tile.TileContext(nc) as tc:
      for _rep in range(repeat):
        with (
            tc.tile_pool(name="consts", bufs=1) as consts,
            tc.tile_pool(name="headt", bufs=1) as headt,
            tc.tile_pool(name="mmps", bufs=4, space="PSUM") as mmps,
            tc.tile_pool(name="stps", bufs=1, space="PSUM") as stps,
            tc.tile_pool(name="onp", bufs=2) as onp,
            tc.tile_pool(name="ptp", bufs=4) as ptp,
            tc.tile_pool(name="rp", bufs=4) as rp,
            tc.tile_pool(name="yp", bufs=2) as ypool,
        ):
            bias_sb = consts.tile([128, 1024], F32)
            nc.sync.dma_start(out=bias_sb, in_=bias[:])
            ident_f = consts.tile([128, 128], F32)
            make_identity(nc, ident_f)
            ident = consts.tile([128, 128], F32R)
            nc.vector.tensor_copy(ident, ident_f)

            NG = PPC // 2
            qht2 = [headt.tile([128, 2048], F32R, tag=f"qht{g}", name=f"qht{g}")
                    for g in range(NG)]
            kht2 = [headt.tile([128, 2048], F32R, tag=f"kht{g}", name=f"kht{g}")
                    for g in range(NG)]
            vh = [headt.tile([128, 16 * 65], F32R, tag=f"vh{p}", name=f"vh{p}")
                  for p in range(PPC)]

            def emit_phases(xt_sb, pwork):
                """Projections + shuffles + transposes for all pairs."""
                for (phase_i, wparam, sh, is_v) in (
                        (0, wq, qsh, False), (1, wk, ksh, False),
                        (2, wv, vsh, True)):
                    with tc.tile_pool(name=f"pw{phase_i}", bufs=1) as pw:
                        w_sb = pw.tile([128, 8, 1024], F32R, tag="w",
                                       name=f"w{phase_i}")
                        for kc in range(8):
                            nc.sync.dma_start(
                                out=w_sb[:, kc, :],
                                in_=wparam.rearrange(
                                    "(c p) j -> p c j", p=128)[:, kc, :])
                      # loop groups inside the weight phase
                        for g2 in range(PPC // 2):
                          if is_v:
                              for ii in range(2):
                                  p = 2 * g2 + ii
                                  nat = pwork.tile([128, 1024], F32R, tag="natv",
                                                   bufs=2, name=f"natv{p}")
                                  for jh in range(2):
                                      ps = mmps.tile([128, 512], F32, tag="mm",
                                                     name="projps")
                                      for kc in range(8):
                                          nc.tensor.matmul(
                                              ps,
                                              lhsT=xt_sb[2 * g2 + ii][:, kc, :],
                                              rhs=w_sb[:, kc,
                                                       jh * 512:(jh + 1) * 512],
                                              start=(kc == 0), stop=(kc == 7),
                                          )
                                      nc.vector.tensor_copy(
                                          nat[:, jh * 512:(jh + 1) * 512], ps)
                                  shr = sh[p].rearrange(
                                      "(il pp2) (t j) -> t il pp2 j",
                                      il=8, t=16)[:, :, :, 0:64]
                                  nc.gpsimd.dma_start(out=shr, in_=nat[:])
                                  nc.sync.dma_start(out=vh[p][:], in_=sh[p])
                                  nc.gpsimd.dma_start(
                                      out=vh[p].rearrange(
                                          "q (b c) -> q b c", c=65)[:, :, 64],
                                      in_=ones[:])  # ones column at 65b+64
                          else:
                              # pair-interleaved (pp, pair, j2) scratch: bounce
                              # write streams 512B-contiguous runs
                              nat2 = pwork.tile([128, 16, 2, 64], F32R, tag="nat",
                                                bufs=3, name=f"nat2_{g2}_{phase_i}")
                              for ii in range(2):
                                  p = 2 * g2 + ii
                                  for jh in range(2):
                                      ps = mmps.tile([128, 512], F32, tag="mm",
                                                     name="projps")
                                      for kc in range(8):
                                          nc.tensor.matmul(
                                              ps,
                                              lhsT=xt_sb[2 * g2 + ii][:, kc, :],
                                              rhs=w_sb[:, kc,
                                                       jh * 512:(jh + 1) * 512],
                                              start=(kc == 0), stop=(kc == 7),
                                          )
                                      nc.vector.tensor_copy(
                                          nat2[:, 8 * jh:8 * (jh + 1), ii, :], ps)
                              shr = sh[g2].rearrange(
                                  "(il pp2) (t w j) -> t il pp2 w j",
                                  il=8, t=16, w=2)
                              nc.gpsimd.dma_start(out=shr, in_=nat2[:])
                              hh2 = pwork.tile([128, 2048], F32R, tag="hh",
                                               bufs=2, name=f"hh{g2}_{phase_i}")
                              nc.sync.dma_start(out=hh2[:], in_=sh[g2])
                              dst = qht2[g2] if phase_i == 0 else kht2[g2]
                              for bt in range(4):
                                  tb = stps.tile([128, 512], F32R, tag="stA",
                                                 name="trps")
                                  for j in range(4):
                                      ti = 4 * bt + j
                                      nc.tensor.transpose(
                                          tb[:, j * 128:(j + 1) * 128],
                                          hh2[:, ti * 128:(ti + 1) * 128],
                                          ident,
                                      )
                                  nc.scalar.copy(
                                      dst[:, bt * 512:(bt + 1) * 512], tb)

            def emit_attention(g):
                onorm2 = onp.tile([128, 2048], F32R, tag="onorm",
                                  name=f"onorm{g}")
                for a in range(4):
                    pvs = [mmps.tile([65, 512], F32, tag="mm",
                                     name=f"pv_{i}") for i in range(2)]
                    for gg in range(2 * a + 2):
                        sts = [stps.tile([128, 1024], F32, tag=t_,
                                         name=f"st{t_}")
                               for t_ in ("stA", "stB")]
                        for q2 in range(2):
                            bb = 2 * gg + q2
                            for i in range(2):
                                nc.tensor.matmul(
                                    sts[i][:, q2 * 512:(q2 + 1) * 512],
                                    lhsT=kht2[g][64 * i:64 * i + 64,
                                                 bb * 128:(bb + 1) * 128],
                                    rhs=qht2[g][64 * i:64 * i + 64,
                                                a * 512:(a + 1) * 512],
                                    start=True, stop=True,
                                )
                        for i in range(2):
                            pt = ptp.tile([128, 1024], F32R, tag="pt",
                                          name=f"pt_{i}")
                            nc.scalar.activation(pt, sts[i], EXP)
                            if gg >= 2 * a:  # diagonal: causal mask
                                r0 = 2 * (gg - 2 * a)
                                nc.gpsimd.affine_select(
                                    out=pt.rearrange("q (w j) -> q w j", w=2),
                                    in_=pt.rearrange("q (w j) -> q w j", w=2),
                                    compare_op=mybir.AluOpType.is_ge,
                                    fill=0.0,
                                    base=-128 * r0,
                                    pattern=[[-128, 2], [1, 512]],
                                    channel_multiplier=-1,
                                )
                            for q2 in range(2):
                                bb = 2 * gg + q2
                                nc.tensor.matmul(
                                    pvs[i],
                                    lhsT=vh[2 * g + i][:, bb * 65:bb * 65 + 65],
                                    rhs=pt[:, q2 * 512:(q2 + 1) * 512],
                                    start=(bb == 0), stop=(bb == 4 * a + 3),
                                )
                    for i in range(2):
                        r1 = rp.tile([1, 512], F32, tag="r1", name="r1_t")
                        nc.vector.reciprocal(r1, pvs[i][64:65, :])
                        rb = rp.tile([64, 512], F32, tag="rb", name="rb_t")
                        nc.gpsimd.partition_broadcast(rb, r1)
                        nc.vector.tensor_mul(
                            onorm2[64 * i:64 * i + 64, a * 512:(a + 1) * 512],
                            pvs[i][0:64, :], rb)

                return onorm2

            def emit_y(g, onorm2, wo_sb):
                # row-packed output projection for both pairs of the group
                onorm_r = onorm2.rearrange("q (i t) -> q t i", t=16)
                ysbs = [ypool.tile([128, 1024], F32, tag="ysb",
                                   name=f"ysb{g}_{i}") for i in range(2)]
                for jh in range(2):
                    yps = [mmps.tile([128, 512], F32, tag="mm",
                                     name=f"ypsum_{i}") for i in range(2)]
                    for t in range(16):
                        for i in range(2):
                            nc.tensor.matmul(
                                yps[i],
                                lhsT=onorm_r[64 * i:64 * i + 64, t, :],
                                rhs=wo_sb[64 * i:64 * i + 64,
                                          t * 1024 + jh * 512:
                                          t * 1024 + (jh + 1) * 512],
                                start=(t == 0), stop=(t == 15),
                            )
                    for i in range(2):
                        nc.vector.tensor_add(
                            ysbs[i][:, jh * 512:(jh + 1) * 512], yps[i],
                            bias_sb[:, jh * 512:(jh + 1) * 512])
                for i in range(2):
                    nc.sync.dma_start(out=out[2 * g + i], in_=ysbs[i])

            # pipeline: group-0 phases; group-1 phases overlap group-0
            # attention (DMA is idle during attention)
            with tile.TileContext.tile_pool(tc, name="xtp", bufs=1) as xtp, \
                 tile.TileContext.tile_pool(tc, name="pwork", bufs=1) as pwork:
                xt_sb = []
                for p in range(PPC):
                    t = xtp.tile([128, 8, 128], F32R, tag=f"xt{p}", name=f"xtsb{p}")
                    nc.scalar.dma_start(
                        out=t, in_=xt[p].rearrange("(c p) i -> p c i", p=128))
                    xt_sb.append(t)
                emit_phases(xt_sb, pwork)

            with tc.tile_pool(name="p2", bufs=1) as p2:
                wo_sb = p2.tile([128, 16 * 1024], F32R, tag="wo")
                for wc in range(8):
                    nc.sync.dma_start(out=wo_sb[0:64, wc * 2048:(wc + 1) * 2048],
                                      in_=wo[:, wc * 2048:(wc + 1) * 2048])
                    nc.vector.tensor_copy(
                        wo_sb[64:128, wc * 2048:(wc + 1) * 2048],
                        wo_sb[0:64, wc * 2048:(wc + 1) * 2048])
                for g in range(NG):
                    onorm2 = emit_attention(g)
                    emit_y(g, onorm2, wo_sb)

    nc.finalize()




    return nc


def _host_prep(input_seq_embs, W_Q, W_K, W_V, W_O, b_O):
    X = np.asarray(input_seq_embs, dtype=np.float32)
    WQ = np.asarray(W_Q, dtype=np.float32)
    WK = np.asarray(W_K, dtype=np.float32)
    WV = np.asarray(W_V, dtype=np.float32)
    WO = np.asarray(W_O, dtype=np.float32)
    bO = np.asarray(b_O, dtype=np.float32)

    wq_arr = np.ascontiguousarray(WQ.T)
    wk_arr = np.ascontiguousarray(WK.T)
    wv_arr = np.ascontiguousarray(WV.T)
    # wo[j2, 1024 t + jo] = W_O.T[64 t + j2, jo]
    wo_arr = np.ascontiguousarray(
        WO.T.reshape(16, 64, 1024).transpose(1, 0, 2).reshape(64, 16 * 1024))
    bias_arr = np.ascontiguousarray(
        np.broadcast_to(bO, (128, 1024)).astype(np.float32))

    in_maps = []
    for c in range(NCORES):
        xts = []
        for p in range(PPC):
            g = PPC * c + p
            bb, hh = g // H, g % H
            xts.append(np.ascontiguousarray(X[bb, 128 * hh:128 * (hh + 1), :].T))
        in_maps.append({
            "xt": np.stack(xts),
            "wq": wq_arr, "wk": wk_arr, "wv": wv_arr, "wo": wo_arr,
            "bias": bias_arr,
            "ones": np.ones((128, 16), dtype=np.float32),
        })
    return in_maps


_CACHED_NC = None


def get_nc():
    global _CACHED_NC
    if _CACHED_NC is None:
        _CACHED_NC = build_nc()
    return _CACHED_NC


def kernel(**inputs) -> np.ndarray:
    nc = get_nc()
    in_maps = _host_prep(**inputs)
    res = run_bass_kernel_spmd(nc, in_maps, list(range(NCORES)))
    out = np.empty((B, L, D), dtype=np.float32)
    for c in range(NCORES):
        y = res.results[c]["out"]  # (4, 128, 1024)
        for p in range(PPC):
            g = PPC * c + p
            bb, hh = g // H, g % H
            out[bb, 128 * hh:128 * (hh + 1), :] = y[p]
    return out



# revision 2
# speedup vs baseline: 7.4599x; 7.4599x over previous
"""Trainium2 Bass kernel for nn_MultiHeadAttention_66322884984909.

Math (faithful to reference):
  Q = X @ W_Q.T reshaped (B, H, L, hd) via DIRECT reshape -> head h owns rows
  128h:128(h+1) of the projected (L, D) matrix, reinterpreted as (L=2048, hd=64).
  Heads are sequence-parallel: 32 (batch, head) pairs, 8 cores x 4 pairs.

v2 design (vs v1):
  - Q/K are computed PRE-TRANSPOSED: Qf^T = W_Q.T' @ X_s^T with output
    partitions = W-output-columns (so no DRAM shuffle bounce and no PE
    transposes).  psum chunks [(t-parity, j), (pair-in-group, r)] are copied
    to qht2/kht2 [64i+j, s=16r+2c+parity] by Pool/DVE strided copies.
  - Causal masking via PSUM PREFILL: diagonal S tiles accumulate on top of a
    constant 0/-1e30 triangle tile (Pool copy), so exp() naturally zeroes the
    masked region -- no affine_select in the attention inner loop.
  - Output projection contracts 128-deep: O is stored as o2[(s-parity, j2),
    (u, r)] so Y needs 8 accumulating matmuls per 512-col chunk, not 16.
  - V path unchanged: X @ W_V.T then DRAM scatter bounce into [s-part,
    block-major (65)] layout with a ones column for softmax row sums.
  - No max-subtraction in softmax: logits ~ N(0, 64); exp stays finite in
    fp32.  All matmuls fp32r (full-rate, free dim >= 256).
"""

import numpy as np

import concourse.bass as bass
from concourse import bacc
import concourse.mybir as mybir
import concourse.tile as tile
from concourse.bass_utils import run_bass_kernel_spmd

F32 = mybir.dt.float32
F32R = mybir.dt.float32r
BF16 = mybir.dt.bfloat16
EXP = mybir.ActivationFunctionType.Exp

B, L, D = 2, 2048, 1024
H, HD = 16, 64
NCORES = 8
PPC = 4   # pairs per core
NG = 2    # groups of 2 pairs
NEG = -1.0e30


def build_nc(repeat=1):
    nc = bacc.Bacc(trn_type="TRN2", target_bir_lowering=False, debug=False)

    # xt[p_, kc*512 + 128*pair + r] = X_pair[r, 128*kc + p_]
    xt = nc.declare_dram_parameter("xt", [128, 8 * PPC * 128], F32R,
                                   isOutput=False)
    # xtb: same layout as xt, bf16 (feeds the V projection)
    xtb = nc.declare_dram_parameter("xtb", [128, 8 * PPC * 128], BF16,
                                    isOutput=False)
    wq = nc.declare_dram_parameter("wq", [1024, 1024], F32R, isOutput=False)
    wk = nc.declare_dram_parameter("wk", [1024, 1024], F32R, isOutput=False)
    wv = nc.declare_dram_parameter("wv", [1024, 1024], BF16, isOutput=False)
    # wo[64*parity + j2, 1024*u + jo] = W_O.T[64*(2u+parity) + j2, jo]
    wo = nc.declare_dram_parameter("wo", [128, 8 * 1024], BF16, isOutput=False)
    bias = nc.declare_dram_parameter("bias", [128, 1024], F32, isOutput=False)
    out = nc.declare_dram_parameter("out", [PPC, 128, 1024], F32, isOutput=True)
    vsh = nc.dram_tensor("vsh", [PPC, 128, 1040], BF16)

    with tile.TileContext(nc) as tc:
      for _rep in range(repeat):
        with (
            tc.tile_pool(name="consts", bufs=1) as consts,
            tc.tile_pool(name="headt", bufs=1) as headt,
            tc.tile_pool(name="mmps", bufs=4, space="PSUM") as mmps,
            tc.tile_pool(name="stps", bufs=1, space="PSUM") as stps,
        ):
            bias_sb = consts.tile([128, 1024], F32)
            nc.scalar.dma_start(out=bias_sb, in_=bias[:])
            # tri01[k, q'] = 1.0 if q' >= k else 0.0 (strip causal mask)
            tri01 = consts.tile([128, 128], BF16)
            nc.gpsimd.memset(tri01, 1.0)
            nc.gpsimd.affine_select(
                out=tri01, in_=tri01,
                compare_op=mybir.AluOpType.is_ge,
                fill=0.0,
                base=0,
                pattern=[[1, 128]],
                channel_multiplier=-1,
            )

            # qht2/kht2[g]: [64*i + j, s] for pair 2g+i  (j = head dim)
            qht2 = [headt.tile([128, 2048], F32R, tag=f"qht{g}", name=f"qht{g}")
                    for g in range(NG)]
            kht2 = [headt.tile([128, 2048], F32R, tag=f"kht{g}", name=f"kht{g}")
                    for g in range(NG)]
            # vh[p]: [s-in-block, 65*bb + j] with ones column at j=64
            vh = [headt.tile([128, 16 * 65], BF16, tag=f"vh{p}", name=f"vh{p}")
                  for p in range(PPC)]

            def qk_chunks(w_sb, xq_sb, dsts, g, wname):
                """Transposed projection, as 4 chunk-closures per (W, g).

                Chunk (half, bank): one psum tile [128, 512] = 2 c-quarters
                (c = 4*half + 2*bank + cq).  Quarter c holds [(parity, j),
                (i, r)] with W-output column 128c + 64*parity + j; copies land
                in dsts[g][64i + j, s = 16r + 2c + parity].  psum start/stop
                act on whole 2KB banks, so only the first quarter starts and
                only the last stops the accumulation group.
                """
                def mk(half):
                    def emit(tag="stA"):
                        pg = stps.tile(
                            [128, 1024], F32, tag=tag,
                            name=f"prj_{wname}_{g}_{half}")
                        for kc in range(8):
                            for c4 in range(4):
                                c = 4 * half + c4
                                # psum start/stop act on whole 2KB banks: a
                                # bank spans two 256-col quarters, so only the
                                # first quarter starts / last quarter stops
                                nc.tensor.matmul(
                                    pg[:, 256 * c4:256 * c4 + 256],
                                    lhsT=w_sb[:, kc, c * 128:(c + 1) * 128],
                                    rhs=xq_sb[:, kc, 2 * g:2 * g + 2, :],
                                    start=(kc == 0 and c4 % 2 == 0),
                                    stop=(kc == 7 and c4 % 2 == 1),
                                )
                        v = dsts[g].rearrange(
                            "(i j) (r c two) -> i j two c r", i=2, c=8, two=2)
                        pv4 = pg.rearrange(
                            "p (c4 two r) -> p c4 two r", c4=4, two=2)
                        for parity in range(2):
                            for i in range(2):
                                nc.vector.tensor_copy(
                                    v[i, :, parity, 4 * half:4 * half + 4, :],
                                    pv4[64 * parity:64 * parity + 64, :, i, :])
                    return emit
                return [mk(h) for h in range(2)]

            def v_chunks(wv_sb, xb_sb, pwork, pairs):
                """V projection + DRAM shuffle bounce, one closure per
                (pair, jh-half)."""
                nats = {}

                def mk(p, jh):
                    def emit(tag=None):
                        if jh == 0:
                            nats[p] = pwork.tile([128, 1024], BF16, tag="natv",
                                                 bufs=2, name=f"natv{p}")
                        nat = nats[p]
                        ps = mmps.tile([128, 512], F32, tag="mm",
                                       name="projps")
                        for kc in range(8):
                            nc.tensor.matmul(
                                ps,
                                lhsT=xb_sb[:, kc, p, :],
                                rhs=wv_sb[:, kc, jh * 512:(jh + 1) * 512],
                                start=(kc == 0), stop=(kc == 7),
                            )
                        nc.vector.tensor_copy(
                            nat[:, jh * 512:(jh + 1) * 512], ps)
                        if jh == 1:
                            shr = vsh[p].rearrange(
                                "(il pp2) (t j) -> t il pp2 j",
                                il=8, t=16)[:, :, :, 0:64]
                            nc.gpsimd.dma_start(out=shr, in_=nat[:])
                            nc.scalar.dma_start(out=vh[p][:], in_=vsh[p])
                            nc.gpsimd.memset(
                                vh[p].rearrange(
                                    "q (b c) -> q b c", c=65)[:, :, 64],
                                1.0)  # ones column at 65b+64
                    return emit
                return [mk(p, jh) for p in pairs for jh in range(2)]

            def emit_attention(g, ptp, rp, o2p, inter=(), sched=(6, 10)):
                """Returns o2 tiles [pair i][(parity, j2), (u, r)].

                Software-pipelined: the PV matmuls for unit n are emitted
                AFTER the S matmuls of unit n+1, so the PE never stalls on
                the exp latency (psum stA/stB rotate at depth 2).
                """
                o2 = [o2p.tile([128, 8 * 128], BF16, tag=f"o2_{i}",
                               name=f"o2_{g}_{i}") for i in range(2)]
                pvs_by_a = {}

                def emit_pv(a, gg, i, pt):
                    diag = gg >= 2 * a
                    d = gg - 2 * a
                    for q2 in range(2):
                        bb = 2 * gg + q2
                        if not diag:
                            nc.tensor.matmul(
                                pvs_by_a[a][i],
                                lhsT=vh[2 * g + i][:, bb * 65:bb * 65 + 65],
                                rhs=pt[:, q2 * 512:(q2 + 1) * 512],
                                start=(bb == 0),
                                stop=(bb == 4 * a + 3),
                            )
                        else:
                            m = 2 * d + q2
                            lo = q2 * 512 + 128 * m
                            nc.tensor.matmul(
                                pvs_by_a[a][i][:, 128 * m:512],
                                lhsT=vh[2 * g + i][:, bb * 65:bb * 65 + 65],
                                rhs=pt[:, lo:(q2 + 1) * 512],
                                start=(bb == 0),
                                stop=(bb == 4 * a + 3),
                                skip_group_check=True,
                            )

                def emit_norm(a):
                    pvs = pvs_by_a.pop(a)
                    for i in range(2):
                        r1 = rp.tile([1, 512], F32, tag="r1", name="r1_t")
                        nc.vector.reciprocal(r1, pvs[i][64:65, :])
                        rb = rp.tile([64, 512], F32, tag="rb", name="rb_t")
                        nc.gpsimd.partition_broadcast(rb, r1)
                        # o2[64*par + j2, 128u + 32a + r'] =
                        #     pvs[j2, 16r' + 2u + par] * rb[...]
                        pv_v = pvs[i][0:64, :].rearrange(
                            "j (rr uu two) -> j two uu rr", two=2, uu=8)
                        rb_v = rb.rearrange(
                            "j (rr uu two) -> j two uu rr", two=2, uu=8)
                        o2_v = o2[i].rearrange(
                            "q (u rr) -> q u rr", u=8)[:, :, 32 * a:32 * a + 32]
                        for par in range(2):
                            nc.vector.tensor_mul(
                                o2_v[64 * par:64 * par + 64],
                                pv_v[:, par], rb_v[:, par])

                pending = None
                inter = list(inter)
                sched = list(sched)
                rot = [0]

                def next_tag():
                    rot[0] += 1
                    return "stA" if rot[0] % 2 == 0 else "stB"

                units = [(a, gg, i) for a in range(4)
                         for gg in range(2 * a + 2) for i in range(2)]
                for ui, (a, gg, i) in enumerate(units):
                    if inter and sched and ui == sched[0]:
                        sched.pop(0)
                        inter.pop(0)(next_tag())
                    if gg == 0 and i == 0:
                        pvs_by_a[a] = [
                            mmps.tile([65, 512], F32, tag="mm",
                                      name=f"pv{a}_{ii}") for ii in range(2)]
                    diag = gg >= 2 * a
                    d = gg - 2 * a
                    sts = stps.tile([128, 1024], F32, tag=next_tag(),
                                    name=f"st{a}_{gg}_{i}")
                    for q2 in range(2):
                        bb = 2 * gg + q2
                        # diagonal blocks m=1,2 only need cols >= 128m (m=3
                        # would drop the free dim under 256 for no gain)
                        m = 2 * d + q2 if diag else 0
                        off = 128 * m if m in (1, 2) else 0
                        nc.tensor.matmul(
                            sts[:, q2 * 512 + off:(q2 + 1) * 512],
                            lhsT=kht2[g][64 * i:64 * i + 64,
                                         bb * 128:(bb + 1) * 128],
                            rhs=qht2[g][64 * i:64 * i + 64,
                                        a * 512 + off:(a + 1) * 512],
                            start=True, stop=True,
                        )
                    pt = ptp.tile([128, 1024], BF16, tag="pt",
                                  name=f"pt_{a}_{gg}_{i}")
                    if not diag:
                        nc.scalar.activation(pt, sts, EXP)
                    else:
                        # exp only the valid columns; zero the boundary
                        # strip's upper triangle with the 0/1 mask
                        for q2 in range(2):
                            m = 2 * d + q2
                            lo = q2 * 512 + 128 * m
                            hi = (q2 + 1) * 512
                            nc.scalar.activation(
                                pt[:, lo:hi], sts[:, lo:hi], EXP)
                            nc.vector.tensor_mul(
                                pt[:, lo:lo + 128],
                                pt[:, lo:lo + 128], tri01)
                    if pending is not None:
                        emit_pv(*pending)
                        pa, pgg, pi, _ = pending
                        if pgg == 2 * pa + 1 and pi == 1:
                            emit_norm(pa)
                    pending = (a, gg, i, pt)
                emit_pv(*pending)
                emit_norm(pending[0])
                return o2

            def emit_y_parts(g, o2, wo_sb, ypool):
                """Returns closures [part_jh0, part_jh1]; each emits half of
                the output projection so it can interleave with the next
                group's attention stream."""
                ysbs = [ypool.tile([128, 1024], F32, tag="ysb",
                                   name=f"ysb{g}_{i}") for i in range(2)]

                def part(jh, tag=None):
                    yps = [mmps.tile([128, 512], F32, tag="mm",
                                     name=f"ypsum_{i}") for i in range(2)]
                    for u in range(8):
                        for i in range(2):
                            nc.tensor.matmul(
                                yps[i],
                                lhsT=o2[i][:, u * 128:(u + 1) * 128],
                                rhs=wo_sb[:, u * 1024 + jh * 512:
                                          u * 1024 + (jh + 1) * 512],
                                start=(u == 0), stop=(u == 7),
                            )
                    for i in range(2):
                        nc.vector.tensor_add(
                            ysbs[i][:, jh * 512:(jh + 1) * 512], yps[i],
                            bias_sb[:, jh * 512:(jh + 1) * 512])
                        nc.sync.dma_start(
                            out=out[2 * g + i][:, jh * 512:(jh + 1) * 512],
                            in_=ysbs[i][:, jh * 512:(jh + 1) * 512])

                return [lambda tag=None: part(0), lambda tag=None: part(1)]

            with (
                tc.tile_pool(name="xtp", bufs=1) as xtp,
                tc.tile_pool(name="wp", bufs=1) as wp,
                tc.tile_pool(name="pwork", bufs=1) as pwork,
                tc.tile_pool(name="p2", bufs=1) as p2,
                tc.tile_pool(name="ptp", bufs=6) as ptp,
                tc.tile_pool(name="rp", bufs=2) as rp,
                tc.tile_pool(name="o2p", bufs=2) as o2p,
                tc.tile_pool(name="yp", bufs=2) as ypool,
            ):
                xq_sb = xtp.tile([128, 8, PPC, 128], F32R, tag="xq",
                                 name="xqsb")
                xv = xt.rearrange("p (kc pr r) -> p kc pr r", kc=8, pr=PPC)
                for kc in range(8):
                    nc.scalar.dma_start(out=xq_sb[:, kc], in_=xv[:, kc])
                xb_sb = xtp.tile([128, 8, PPC, 128], BF16, tag="xb",
                                 name="xbsb")
                nc.scalar.dma_start(out=xb_sb, in_=xtb.rearrange(
                    "p (kc pr r) -> p kc pr r", kc=8, pr=PPC))

                w_sbs = []
                for wi, (wparam, dt_) in enumerate(
                        ((wq, F32R), (wv, BF16), (wk, F32R))):
                    w_sb = wp.tile([128, 8, 1024], dt_, tag=f"w{wi}",
                                   name=f"w{wi}")
                    for kc in range(8):
                        nc.sync.dma_start(
                            out=w_sb[:, kc, :],
                            in_=wparam.rearrange(
                                "(c p) j -> p c j", p=128)[:, kc, :])
                    w_sbs.append(w_sb)
                wo_sb = p2.tile([128, 8 * 1024], BF16, tag="wo")
                nc.sync.dma_start(out=wo_sb, in_=wo[:])

                # group 0 runs as early as possible; group 1's projections
                # and pairs 2/3's V path are injected into attention(g0)'s
                # ACT-bound stream at psum-quiet unit indices
                pre_rot = [0]

                def pre_tag():
                    pre_rot[0] += 1
                    return "stA" if pre_rot[0] % 2 == 0 else "stB"

                q0 = qk_chunks(w_sbs[0], xq_sb, qht2, 0, "q")
                q1 = qk_chunks(w_sbs[0], xq_sb, qht2, 1, "q")
                k0 = qk_chunks(w_sbs[2], xq_sb, kht2, 0, "k")
                k1 = qk_chunks(w_sbs[2], xq_sb, kht2, 1, "k")
                for ch in (q0[0], q1[0], q0[1], q1[1]):
                    ch(pre_tag())
                for ch in v_chunks(w_sbs[1], xb_sb, pwork, [0, 1]):
                    ch()
                for ch in (k0[0], k1[0], k0[1], k1[1]):
                    ch(pre_tag())
                for ch in v_chunks(w_sbs[1], xb_sb, pwork, [2, 3]):
                    ch()

                o2 = emit_attention(
                    g=0, ptp=ptp, rp=rp, o2p=o2p, inter=(), sched=())
                parts = emit_y_parts(0, o2, wo_sb, ypool)
                o2 = emit_attention(g=1, ptp=ptp, rp=rp, o2p=o2p,
                                    inter=parts, sched=(6, 10))
                parts = emit_y_parts(1, o2, wo_sb, ypool)
                for part in parts:
                    part()

    nc.finalize()
    return nc


def _host_prep(input_seq_embs, W_Q, W_K, W_V, W_O, b_O):
    X = np.asarray(input_seq_embs, dtype=np.float32)
    WQ = np.asarray(W_Q, dtype=np.float32)
    WK = np.asarray(W_K, dtype=np.float32)
    WV = np.asarray(W_V, dtype=np.float32)
    WO = np.asarray(W_O, dtype=np.float32)
    bO = np.asarray(b_O, dtype=np.float32)

    import ml_dtypes
    bf16 = ml_dtypes.bfloat16

    wq_arr = np.ascontiguousarray(WQ.T)
    wk_arr = np.ascontiguousarray(WK.T)
    wv_arr = np.ascontiguousarray(WV.T).astype(bf16)
    # wo[64*parity + j2, 1024*u + jo] = W_O.T[64*(2u+parity) + j2, jo]
    wo_arr = np.ascontiguousarray(
        WO.T.reshape(8, 2, 64, 1024).transpose(1, 2, 0, 3).reshape(
            128, 8192)).astype(bf16)
    bias_arr = np.ascontiguousarray(
        np.broadcast_to(bO, (128, 1024)).astype(np.float32))

    in_maps = []
    for c in range(NCORES):
        # xt[p_, kc, pair, r] = X_pair[r, 128*kc + p_]
        xts = np.empty((128, 8, PPC, 128), dtype=np.float32)
        for p in range(PPC):
            g = PPC * c + p
            bb, hh = g // H, g % H
            Xs = X[bb, 128 * hh:128 * (hh + 1), :]      # (128 r, 1024 cin)
            xts[:, :, p, :] = Xs.T.reshape(8, 128, 128).transpose(1, 0, 2)
        xt_arr = np.ascontiguousarray(xts.reshape(128, 8 * PPC * 128))
        in_maps.append({
            "xt": xt_arr, "xtb": xt_arr.astype(bf16),
            "wq": wq_arr, "wk": wk_arr, "wv": wv_arr, "wo": wo_arr,
            "bias": bias_arr,
        })
    return in_maps


_CACHED_NC = None


def get_nc():
    global _CACHED_NC
    if _CACHED_NC is None:
        _CACHED_NC = build_nc()
    return _CACHED_NC


def kernel(**inputs) -> np.ndarray:
    nc = get_nc()
    in_maps = _host_prep(**inputs)
    res = run_bass_kernel_spmd(nc, in_maps, list(range(NCORES)))
    out = np.empty((B, L, D), dtype=np.float32)
    for c in range(NCORES):
        y = res.results[c]["out"]  # (4, 128, 1024)
        for p in range(PPC):
            g = PPC * c + p
            bb, hh = g // H, g % H
            out[bb, 128 * hh:128 * (hh + 1), :] = y[p]
    return out


# revision 3
# speedup vs baseline: 7.5023x; 1.0057x over previous
"""Trainium2 Bass kernel for nn_MultiHeadAttention_66322884984909.

Math (faithful to reference):
  Q = X @ W_Q.T reshaped (B, H, L, hd) via DIRECT reshape -> head h owns rows
  128h:128(h+1) of the projected (L, D) matrix, reinterpreted as (L=2048, hd=64).
  Heads are sequence-parallel: 32 (batch, head) pairs, 8 cores x 4 pairs.

v2 design (vs v1):
  - Q/K are computed PRE-TRANSPOSED: Qf^T = W_Q.T' @ X_s^T with output
    partitions = W-output-columns (so no DRAM shuffle bounce and no PE
    transposes).  psum chunks [(t-parity, j), (pair-in-group, r)] are copied
    to qht2/kht2 [64i+j, s=16r+2c+parity] by Pool/DVE strided copies.
  - Causal masking via PSUM PREFILL: diagonal S tiles accumulate on top of a
    constant 0/-1e30 triangle tile (Pool copy), so exp() naturally zeroes the
    masked region -- no affine_select in the attention inner loop.
  - Output projection contracts 128-deep: O is stored as o2[(s-parity, j2),
    (u, r)] so Y needs 8 accumulating matmuls per 512-col chunk, not 16.
  - V path unchanged: X @ W_V.T then DRAM scatter bounce into [s-part,
    block-major (65)] layout with a ones column for softmax row sums.
  - No max-subtraction in softmax: logits ~ N(0, 64); exp stays finite in
    fp32.  All matmuls fp32r (full-rate, free dim >= 256).
"""

import numpy as np

import concourse.bass as bass
from concourse import bacc
import concourse.mybir as mybir
import concourse.tile as tile
from concourse.bass_utils import run_bass_kernel_spmd

F32 = mybir.dt.float32
F32R = mybir.dt.float32r
BF16 = mybir.dt.bfloat16
EXP = mybir.ActivationFunctionType.Exp

B, L, D = 2, 2048, 1024
H, HD = 16, 64
NCORES = 8
PPC = 4   # pairs per core
NG = 2    # groups of 2 pairs
NEG = -1.0e30


def build_nc(repeat=1):
    nc = bacc.Bacc(trn_type="TRN2", target_bir_lowering=False, debug=False)

    # xt[p_, kc*512 + 128*pair + r] = X_pair[r, 128*kc + p_]
    xt = nc.declare_dram_parameter("xt", [128, 8 * PPC * 128], F32R,
                                   isOutput=False)
    # xtb: same layout as xt, bf16 (feeds the V projection)
    xtb = nc.declare_dram_parameter("xtb", [128, 8 * PPC * 128], BF16,
                                    isOutput=False)
    wq = nc.declare_dram_parameter("wq", [1024, 1024], F32R, isOutput=False)
    wk = nc.declare_dram_parameter("wk", [1024, 1024], F32R, isOutput=False)
    wv = nc.declare_dram_parameter("wv", [1024, 1024], BF16, isOutput=False)
    # wo[64*parity + j2, 1024*u + jo] = W_O.T[64*(2u+parity) + j2, jo]
    wo = nc.declare_dram_parameter("wo", [128, 8 * 1024], BF16, isOutput=False)
    bias = nc.declare_dram_parameter("bias", [128, 1024], F32, isOutput=False)
    out = nc.declare_dram_parameter("out", [PPC, 128, 1024], F32, isOutput=True)
    vsh = nc.dram_tensor("vsh", [PPC, 128, 1040], BF16)

    with tile.TileContext(nc) as tc:
      for _rep in range(repeat):
        with (
            tc.tile_pool(name="consts", bufs=1) as consts,
            tc.tile_pool(name="headt", bufs=1) as headt,
            tc.tile_pool(name="mmps", bufs=4, space="PSUM") as mmps,
            tc.tile_pool(name="stps", bufs=1, space="PSUM") as stps,
        ):
            bias_sb = consts.tile([128, 1024], F32)
            nc.scalar.dma_start(out=bias_sb, in_=bias[:])
            # tri01[k, q'] = 1.0 if q' >= k else 0.0 (strip causal mask)
            tri01 = consts.tile([128, 128], BF16)
            nc.gpsimd.memset(tri01, 1.0)
            nc.gpsimd.affine_select(
                out=tri01, in_=tri01,
                compare_op=mybir.AluOpType.is_ge,
                fill=0.0,
                base=0,
                pattern=[[1, 128]],
                channel_multiplier=-1,
            )

            # qht2/kht2[g]: [64*i + j, s] for pair 2g+i  (j = head dim)
            qht2 = [headt.tile([128, 2048], F32R, tag=f"qht{g}", name=f"qht{g}")
                    for g in range(NG)]
            kht2 = [headt.tile([128, 2048], F32R, tag=f"kht{g}", name=f"kht{g}")
                    for g in range(NG)]
            # vh[p]: [s-in-block, 65*bb + j] with ones column at j=64
            vh = [headt.tile([128, 16 * 65], BF16, tag=f"vh{p}", name=f"vh{p}")
                  for p in range(PPC)]

            def qk_chunks(w_sb, xq_sb, dsts, g, wname):
                """Transposed projection, as 4 chunk-closures per (W, g).

                Chunk (half, bank): one psum tile [128, 512] = 2 c-quarters
                (c = 4*half + 2*bank + cq).  Quarter c holds [(parity, j),
                (i, r)] with W-output column 128c + 64*parity + j; copies land
                in dsts[g][64i + j, s = 16r + 2c + parity].  psum start/stop
                act on whole 2KB banks, so only the first quarter starts and
                only the last stops the accumulation group.
                """
                def mk(half):
                    def emit(tag="stA"):
                        pg = stps.tile(
                            [128, 1024], F32, tag=tag,
                            name=f"prj_{wname}_{g}_{half}")
                        for kc in range(8):
                            for c4 in range(4):
                                c = 4 * half + c4
                                # psum start/stop act on whole 2KB banks: a
                                # bank spans two 256-col quarters, so only the
                                # first quarter starts / last quarter stops
                                nc.tensor.matmul(
                                    pg[:, 256 * c4:256 * c4 + 256],
                                    lhsT=w_sb[:, kc, c * 128:(c + 1) * 128],
                                    rhs=xq_sb[:, kc, 2 * g:2 * g + 2, :],
                                    start=(kc == 0 and c4 % 2 == 0),
                                    stop=(kc == 7 and c4 % 2 == 1),
                                )
                        v = dsts[g].rearrange(
                            "(i j) (r c two) -> i j two c r", i=2, c=8, two=2)
                        pv4 = pg.rearrange(
                            "p (c4 two r) -> p c4 two r", c4=4, two=2)
                        for parity in range(2):
                            for i in range(2):
                                nc.vector.tensor_copy(
                                    v[i, :, parity, 4 * half:4 * half + 4, :],
                                    pv4[64 * parity:64 * parity + 64, :, i, :])
                    return emit
                return [mk(h) for h in range(2)]

            def v_chunks(wv_sb, xb_sb, pwork, pairs):
                """V projection + DRAM shuffle bounce, one closure per
                (pair, jh-half)."""
                nats = {}

                def mk(p, jh):
                    def emit(tag=None):
                        if jh == 0:
                            nats[p] = pwork.tile([128, 1024], BF16, tag="natv",
                                                 bufs=2, name=f"natv{p}")
                        nat = nats[p]
                        ps = mmps.tile([128, 512], F32, tag="mm",
                                       name="projps")
                        for kc in range(8):
                            nc.tensor.matmul(
                                ps,
                                lhsT=xb_sb[:, kc, p, :],
                                rhs=wv_sb[:, kc, jh * 512:(jh + 1) * 512],
                                start=(kc == 0), stop=(kc == 7),
                            )
                        nc.vector.tensor_copy(
                            nat[:, jh * 512:(jh + 1) * 512], ps)
                        if jh == 1:
                            shr = vsh[p].rearrange(
                                "(il pp2) (t j) -> t il pp2 j",
                                il=8, t=16)[:, :, :, 0:64]
                            nc.gpsimd.dma_start(out=shr, in_=nat[:])
                            nc.scalar.dma_start(out=vh[p][:], in_=vsh[p])
                            nc.gpsimd.memset(
                                vh[p].rearrange(
                                    "q (b c) -> q b c", c=65)[:, :, 64],
                                1.0)  # ones column at 65b+64
                    return emit
                return [mk(p, jh) for p in pairs for jh in range(2)]

            def emit_attention(g, ptp, rp, o2p, inter=(), sched=(6, 10)):
                """Returns o2 tiles [pair i][(parity, j2), (u, r)].

                Software-pipelined: the PV matmuls for unit n are emitted
                AFTER the S matmuls of unit n+1, so the PE never stalls on
                the exp latency (psum stA/stB rotate at depth 2).
                """
                o2 = [o2p.tile([128, 8 * 128], BF16, tag=f"o2_{i}",
                               name=f"o2_{g}_{i}") for i in range(2)]
                pvs_by_a = {}

                def emit_pv(a, gg, i, pt):
                    diag = gg >= 2 * a
                    d = gg - 2 * a
                    for q2 in range(2):
                        bb = 2 * gg + q2
                        if not diag:
                            nc.tensor.matmul(
                                pvs_by_a[a][i],
                                lhsT=vh[2 * g + i][:, bb * 65:bb * 65 + 65],
                                rhs=pt[:, q2 * 512:(q2 + 1) * 512],
                                start=(bb == 0),
                                stop=(bb == 4 * a + 3),
                            )
                        else:
                            m = 2 * d + q2
                            lo = q2 * 512 + 128 * m
                            nc.tensor.matmul(
                                pvs_by_a[a][i][:, 128 * m:512],
                                lhsT=vh[2 * g + i][:, bb * 65:bb * 65 + 65],
                                rhs=pt[:, lo:(q2 + 1) * 512],
                                start=(bb == 0),
                                stop=(bb == 4 * a + 3),
                                skip_group_check=True,
                            )

                def emit_norm(a):
                    pvs = pvs_by_a.pop(a)
                    for i in range(2):
                        r1 = rp.tile([1, 512], F32, tag="r1", name="r1_t")
                        nc.vector.reciprocal(r1, pvs[i][64:65, :])
                        rb = rp.tile([64, 512], F32, tag="rb", name="rb_t")
                        nc.gpsimd.partition_broadcast(rb, r1)
                        # o2[64*par + j2, 128u + 32a + r'] =
                        #     pvs[j2, 16r' + 2u + par] * rb[...]
                        pv_v = pvs[i][0:64, :].rearrange(
                            "j (rr uu two) -> j two uu rr", two=2, uu=8)
                        rb_v = rb.rearrange(
                            "j (rr uu two) -> j two uu rr", two=2, uu=8)
                        o2_v = o2[i].rearrange(
                            "q (u rr) -> q u rr", u=8)[:, :, 32 * a:32 * a + 32]
                        for par in range(2):
                            nc.vector.tensor_mul(
                                o2_v[64 * par:64 * par + 64],
                                pv_v[:, par], rb_v[:, par])

                pending = None
                inter = list(inter)
                sched = list(sched)
                rot = [0]

                def next_tag():
                    rot[0] += 1
                    return "stA" if rot[0] % 2 == 0 else "stB"

                units = [(a, gg, i) for a in range(4)
                         for gg in range(2 * a + 2) for i in range(2)]
                for ui, (a, gg, i) in enumerate(units):
                    if inter and sched and ui == sched[0]:
                        sched.pop(0)
                        inter.pop(0)(next_tag())
                    if gg == 0 and i == 0:
                        pvs_by_a[a] = [
                            mmps.tile([65, 512], F32, tag="mm",
                                      name=f"pv{a}_{ii}") for ii in range(2)]
                    diag = gg >= 2 * a
                    d = gg - 2 * a
                    sts = stps.tile([128, 1024], F32, tag=next_tag(),
                                    name=f"st{a}_{gg}_{i}")
                    for q2 in range(2):
                        bb = 2 * gg + q2
                        # diagonal blocks m=1,2 only need cols >= 128m (m=3
                        # would drop the free dim under 256 for no gain)
                        m = 2 * d + q2 if diag else 0
                        off = 128 * m if m in (1, 2) else 0
                        nc.tensor.matmul(
                            sts[:, q2 * 512 + off:(q2 + 1) * 512],
                            lhsT=kht2[g][64 * i:64 * i + 64,
                                         bb * 128:(bb + 1) * 128],
                            rhs=qht2[g][64 * i:64 * i + 64,
                                        a * 512 + off:(a + 1) * 512],
                            start=True, stop=True,
                        )
                    pt = ptp.tile([128, 1024], BF16, tag="pt",
                                  name=f"pt_{a}_{gg}_{i}")
                    if not diag:
                        nc.scalar.activation(pt, sts, EXP)
                    else:
                        # exp only the valid columns; zero the boundary
                        # strip's upper triangle with the 0/1 mask
                        for q2 in range(2):
                            m = 2 * d + q2
                            lo = q2 * 512 + 128 * m
                            hi = (q2 + 1) * 512
                            nc.scalar.activation(
                                pt[:, lo:hi], sts[:, lo:hi], EXP)
                            nc.vector.tensor_mul(
                                pt[:, lo:lo + 128],
                                pt[:, lo:lo + 128], tri01)
                    if pending is not None:
                        emit_pv(*pending)
                        pa, pgg, pi, _ = pending
                        if pgg == 2 * pa + 1 and pi == 1:
                            emit_norm(pa)
                    pending = (a, gg, i, pt)
                emit_pv(*pending)
                emit_norm(pending[0])
                return o2

            def emit_y_parts(g, o2, wo_sb, ypool):
                """Returns closures [part_jh0, part_jh1]; each emits half of
                the output projection so it can interleave with the next
                group's attention stream."""
                ysbs = [ypool.tile([128, 1024], F32, tag="ysb",
                                   name=f"ysb{g}_{i}") for i in range(2)]

                def part(i, tag=None):
                    for jh in range(2):
                        yps = mmps.tile([128, 512], F32, tag="mm",
                                        name=f"ypsum_{i}_{jh}")
                        for u in range(8):
                            nc.tensor.matmul(
                                yps,
                                lhsT=o2[i][:, u * 128:(u + 1) * 128],
                                rhs=wo_sb[:, u * 1024 + jh * 512:
                                          u * 1024 + (jh + 1) * 512],
                                start=(u == 0), stop=(u == 7),
                            )
                        nc.vector.tensor_add(
                            ysbs[i][:, jh * 512:(jh + 1) * 512], yps,
                            bias_sb[:, jh * 512:(jh + 1) * 512])
                        nc.sync.dma_start(
                            out=out[2 * g + i][:, jh * 512:(jh + 1) * 512],
                            in_=ysbs[i][:, jh * 512:(jh + 1) * 512])

                return [lambda tag=None: part(0), lambda tag=None: part(1)]

            with (
                tc.tile_pool(name="xtp", bufs=1) as xtp,
                tc.tile_pool(name="wp", bufs=1) as wp,
                tc.tile_pool(name="pwork", bufs=1) as pwork,
                tc.tile_pool(name="p2", bufs=1) as p2,
                tc.tile_pool(name="ptp", bufs=6) as ptp,
                tc.tile_pool(name="rp", bufs=2) as rp,
                tc.tile_pool(name="o2p", bufs=2) as o2p,
                tc.tile_pool(name="yp", bufs=2) as ypool,
            ):
                xq_sb = xtp.tile([128, 8, PPC, 128], F32R, tag="xq",
                                 name="xqsb")
                xv = xt.rearrange("p (kc pr r) -> p kc pr r", kc=8, pr=PPC)
                for kc in range(8):
                    nc.scalar.dma_start(out=xq_sb[:, kc], in_=xv[:, kc])
                xb_sb = xtp.tile([128, 8, PPC, 128], BF16, tag="xb",
                                 name="xbsb")
                nc.scalar.dma_start(out=xb_sb, in_=xtb.rearrange(
                    "p (kc pr r) -> p kc pr r", kc=8, pr=PPC))

                w_sbs = []
                for wi, (wparam, dt_) in enumerate(
                        ((wq, F32R), (wv, BF16), (wk, F32R))):
                    w_sb = wp.tile([128, 8, 1024], dt_, tag=f"w{wi}",
                                   name=f"w{wi}")
                    for kc in range(8):
                        nc.sync.dma_start(
                            out=w_sb[:, kc, :],
                            in_=wparam.rearrange(
                                "(c p) j -> p c j", p=128)[:, kc, :])
                    w_sbs.append(w_sb)
                wo_sb = p2.tile([128, 8 * 1024], BF16, tag="wo")
                nc.sync.dma_start(out=wo_sb, in_=wo[:])

                # group 0 runs as early as possible; group 1's projections
                # and pairs 2/3's V path are injected into attention(g0)'s
                # ACT-bound stream at psum-quiet unit indices
                pre_rot = [0]

                def pre_tag():
                    pre_rot[0] += 1
                    return "stA" if pre_rot[0] % 2 == 0 else "stB"

                q0 = qk_chunks(w_sbs[0], xq_sb, qht2, 0, "q")
                q1 = qk_chunks(w_sbs[0], xq_sb, qht2, 1, "q")
                k0 = qk_chunks(w_sbs[2], xq_sb, kht2, 0, "k")
                k1 = qk_chunks(w_sbs[2], xq_sb, kht2, 1, "k")
                for ch in (q0[0], q1[0], q0[1], q1[1]):
                    ch(pre_tag())
                for ch in v_chunks(w_sbs[1], xb_sb, pwork, [0, 1]):
                    ch()
                for ch in (k0[0], k1[0], k0[1], k1[1]):
                    ch(pre_tag())
                for ch in v_chunks(w_sbs[1], xb_sb, pwork, [2, 3]):
                    ch()

                o2 = emit_attention(
                    g=0, ptp=ptp, rp=rp, o2p=o2p, inter=(), sched=())
                parts = emit_y_parts(0, o2, wo_sb, ypool)
                o2 = emit_attention(g=1, ptp=ptp, rp=rp, o2p=o2p,
                                    inter=parts, sched=(6, 10))
                parts = emit_y_parts(1, o2, wo_sb, ypool)
                for part in parts:
                    part()

    nc.finalize()
    return nc


def _host_prep(input_seq_embs, W_Q, W_K, W_V, W_O, b_O):
    X = np.asarray(input_seq_embs, dtype=np.float32)
    WQ = np.asarray(W_Q, dtype=np.float32)
    WK = np.asarray(W_K, dtype=np.float32)
    WV = np.asarray(W_V, dtype=np.float32)
    WO = np.asarray(W_O, dtype=np.float32)
    bO = np.asarray(b_O, dtype=np.float32)

    import ml_dtypes
    bf16 = ml_dtypes.bfloat16

    wq_arr = np.ascontiguousarray(WQ.T)
    wk_arr = np.ascontiguousarray(WK.T)
    wv_arr = np.ascontiguousarray(WV.T).astype(bf16)
    # wo[64*parity + j2, 1024*u + jo] = W_O.T[64*(2u+parity) + j2, jo]
    wo_arr = np.ascontiguousarray(
        WO.T.reshape(8, 2, 64, 1024).transpose(1, 2, 0, 3).reshape(
            128, 8192)).astype(bf16)
    bias_arr = np.ascontiguousarray(
        np.broadcast_to(bO, (128, 1024)).astype(np.float32))

    in_maps = []
    for c in range(NCORES):
        # xt[p_, kc, pair, r] = X_pair[r, 128*kc + p_]
        xts = np.empty((128, 8, PPC, 128), dtype=np.float32)
        for p in range(PPC):
            g = PPC * c + p
            bb, hh = g // H, g % H
            Xs = X[bb, 128 * hh:128 * (hh + 1), :]      # (128 r, 1024 cin)
            xts[:, :, p, :] = Xs.T.reshape(8, 128, 128).transpose(1, 0, 2)
        xt_arr = np.ascontiguousarray(xts.reshape(128, 8 * PPC * 128))
        in_maps.append({
            "xt": xt_arr, "xtb": xt_arr.astype(bf16),
            "wq": wq_arr, "wk": wk_arr, "wv": wv_arr, "wo": wo_arr,
            "bias": bias_arr,
        })
    return in_maps


_CACHED_NC = None


def get_nc():
    global _CACHED_NC
    if _CACHED_NC is None:
        _CACHED_NC = build_nc()
    return _CACHED_NC


def kernel(**inputs) -> np.ndarray:
    nc = get_nc()
    in_maps = _host_prep(**inputs)
    res = run_bass_kernel_spmd(nc, in_maps, list(range(NCORES)))
    out = np.empty((B, L, D), dtype=np.float32)
    for c in range(NCORES):
        y = res.results[c]["out"]  # (4, 128, 1024)
        for p in range(PPC):
            g = PPC * c + p
            bb, hh = g // H, g % H
            out[bb, 128 * hh:128 * (hh + 1), :] = y[p]
    return out


# revision 5
# speedup vs baseline: 7.5972x; 1.0127x over previous
"""Trainium2 Bass kernel for nn_MultiHeadAttention_66322884984909.

Math (faithful to reference):
  Q = X @ W_Q.T reshaped (B, H, L, hd) via DIRECT reshape -> head h owns rows
  128h:128(h+1) of the projected (L, D) matrix, reinterpreted as (L=2048, hd=64).
  Heads are sequence-parallel: 32 (batch, head) pairs, 8 cores x 4 pairs.

Design:
  - Q/K computed PRE-TRANSPOSED: Qf^T = W_Q.T' @ X_s^T with output partitions
    = W-output-columns, batching a group's 2 pairs in the matmul free dim.
    psum quarters [(t-parity, j), (pair, r)] land in qht2/kht2[64i+j,
    s=16r+2c+parity] via strided DVE copies -- no DRAM shuffle bounce and no
    PE transposes for Q/K.  (psum start/stop act on whole 2KB banks: only the
    first/last quarter of a bank starts/stops its accumulation group.)
  - Causal masking without an inner-loop affine_select: on diagonal S tiles,
    exp and P@V read only columns q >= 128m of each key block; the 128-wide
    boundary strip is zeroed by one bf16 DVE multiply with a constant 0/1
    triangle.  The m=0 block is full width, so the psum accumulation group
    start stays uniform (bb == 0).
  - Attention is software-pipelined per unit (a-chunk, key-block-pair, pair):
    S(n+1) is emitted BEFORE PV(n) so the PE never idles on the exp latency;
    sts psum tiles double-buffer via a shared stA/stB tag rotation.
    Softmax row sums ride along as a 65th ones-column of V; normalization
    (reciprocal + partition_broadcast + strided muls) scatters O directly
    into the o2 layout [(s-parity, j2), (u, r)].
  - Output projection contracts 128-deep (8 accumulating matmuls per 512-col
    half), split into per-pair closures interleaved into group 1's
    attention stream.  Both groups' attention form ONE fused pipeline, so
    group 0's drain overlaps group 1's ramp-up.  W_V loads first and the
    descriptor-heavy V shuffle-scatter overlaps the W_Q/W_K streams;
    dummy matmuls/exp ramp the PE p-state and activation table during the
    initial DMA wait (kc is the outer loop so matmuls chase the W DMAs).
  - bf16 on the linear-only paths (W_V + X-for-V, the V DRAM bounce, exp
    output P, O, W_O): halves their DMA/SBUF cost with ~3e-3 relative error
    (the exp-amplified Q/K path stays fp32r).
  - No max-subtraction in softmax: logits ~ N(0, 64); exp stays finite in
    fp32.  All fp32 matmuls run as fp32r with free dim >= 256 (full rate).

Cost-model (TimelineSim) total: 170.7 us vs 248.9 us for the v1 baseline.
"""

import numpy as np

import concourse.bass as bass
from concourse import bacc
import concourse.mybir as mybir
import concourse.tile as tile
from concourse.bass_utils import run_bass_kernel_spmd

F32 = mybir.dt.float32
F32R = mybir.dt.float32r
BF16 = mybir.dt.bfloat16
EXP = mybir.ActivationFunctionType.Exp

B, L, D = 2, 2048, 1024
H, HD = 16, 64
NCORES = 8
PPC = 4   # pairs per core
NG = 2    # groups of 2 pairs
NEG = -1.0e30


def build_nc(repeat=1):
    nc = bacc.Bacc(trn_type="TRN2", target_bir_lowering=False, debug=False)

    # xt[p_, kc*512 + 128*pair + r] = X_pair[r, 128*kc + p_]
    xt = nc.declare_dram_parameter("xt", [128, 8 * PPC * 128], F32R,
                                   isOutput=False)
    # xtb: same layout as xt, bf16 (feeds the V projection)
    xtb = nc.declare_dram_parameter("xtb", [128, 8 * PPC * 128], BF16,
                                    isOutput=False)
    wq = nc.declare_dram_parameter("wq", [1024, 1024], F32R, isOutput=False)
    wk = nc.declare_dram_parameter("wk", [1024, 1024], F32R, isOutput=False)
    wv = nc.declare_dram_parameter("wv", [1024, 1024], BF16, isOutput=False)
    # wo[64*parity + j2, 1024*u + jo] = W_O.T[64*(2u+parity) + j2, jo]
    wo = nc.declare_dram_parameter("wo", [128, 8 * 1024], BF16, isOutput=False)
    bias = nc.declare_dram_parameter("bias", [128, 1024], F32, isOutput=False)
    out = nc.declare_dram_parameter("out", [PPC, 128, 1024], F32, isOutput=True)
    vsh = nc.dram_tensor("vsh", [PPC, 128, 1040], BF16)

    with tile.TileContext(nc) as tc:
      for _rep in range(repeat):
        with (
            tc.tile_pool(name="consts", bufs=1) as consts,
            tc.tile_pool(name="headt", bufs=1) as headt,
            tc.tile_pool(name="mmps", bufs=4, space="PSUM") as mmps,
            tc.tile_pool(name="stps", bufs=1, space="PSUM") as stps,
        ):
            bias_sb = consts.tile([128, 1024], F32)
            # tri01[k, q'] = 1.0 if q' >= k else 0.0 (strip causal mask)
            tri01 = consts.tile([128, 128], BF16)
            nc.gpsimd.memset(tri01, 1.0)
            # dummy exp to preload the activation table during the
            # projection phase (the lazy load costs 1.3us otherwise)
            warm = consts.tile([128, 1], F32, tag="warm")
            nc.gpsimd.memset(warm, 0.0)
            nc.scalar.activation(warm, warm, EXP)
            # dummy matmuls ramp the PE p-state (full clock needs ~3us of
            # continuous execution) while the first weight DMAs stream in
            wmm = consts.tile([128, 256], F32, tag="wmm")
            nc.gpsimd.memset(wmm, 0.0)
            for _w in range(6):
                pw = mmps.tile([128, 256], F32, tag="mm", name=f"warmmm{_w}")
                nc.tensor.matmul(pw, lhsT=wmm[:, 0:128], rhs=wmm,
                                 start=True, stop=True)
            nc.gpsimd.affine_select(
                out=tri01, in_=tri01,
                compare_op=mybir.AluOpType.is_ge,
                fill=0.0,
                base=0,
                pattern=[[1, 128]],
                channel_multiplier=-1,
            )

            # qht2/kht2[g]: [64*i + j, s] for pair 2g+i  (j = head dim)
            qht2 = [headt.tile([128, 2048], F32R, tag=f"qht{g}", name=f"qht{g}")
                    for g in range(NG)]
            kht2 = [headt.tile([128, 2048], F32R, tag=f"kht{g}", name=f"kht{g}")
                    for g in range(NG)]
            # vh[p]: [s-in-block, 65*bb + j] with ones column at j=64
            vh = [headt.tile([128, 16 * 65], BF16, tag=f"vh{p}", name=f"vh{p}")
                  for p in range(PPC)]

            def qk_chunks(w_sb, xq_sb, dsts, g, wname):
                """Transposed projection, as 4 chunk-closures per (W, g).

                Chunk (half, bank): one psum tile [128, 512] = 2 c-quarters
                (c = 4*half + 2*bank + cq).  Quarter c holds [(parity, j),
                (i, r)] with W-output column 128c + 64*parity + j; copies land
                in dsts[g][64i + j, s = 16r + 2c + parity].  psum start/stop
                act on whole 2KB banks, so only the first quarter starts and
                only the last stops the accumulation group.
                """
                def mk(half):
                    def emit(tag="stA"):
                        pg = stps.tile(
                            [128, 1024], F32, tag=tag,
                            name=f"prj_{wname}_{g}_{half}")
                        for kc in range(8):
                            for c4 in range(4):
                                c = 4 * half + c4
                                # psum start/stop act on whole 2KB banks: a
                                # bank spans two 256-col quarters, so only the
                                # first quarter starts / last quarter stops
                                nc.tensor.matmul(
                                    pg[:, 256 * c4:256 * c4 + 256],
                                    lhsT=w_sb[:, kc, c * 128:(c + 1) * 128],
                                    rhs=xq_sb[:, kc, 2 * g:2 * g + 2, :],
                                    start=(kc == 0 and c4 % 2 == 0),
                                    stop=(kc == 7 and c4 % 2 == 1),
                                )
                        v = dsts[g].rearrange(
                            "(i j) (r c two) -> i j two c r", i=2, c=8, two=2)
                        pv4 = pg.rearrange(
                            "p (c4 two r) -> p c4 two r", c4=4, two=2)
                        for parity in range(2):
                            for i in range(2):
                                nc.vector.tensor_copy(
                                    v[i, :, parity, 4 * half:4 * half + 4, :],
                                    pv4[64 * parity:64 * parity + 64, :, i, :])
                    return emit
                return [mk(h) for h in range(2)]

            def v_chunks(wv_sb, xb_sb, pwork, pairs):
                """V projection + DRAM shuffle bounce, one closure per
                (pair, jh-half)."""
                nats = {}

                def mk(p, jh):
                    def emit(tag=None):
                        if jh == 0:
                            nats[p] = pwork.tile([128, 1024], BF16, tag="natv",
                                                 bufs=2, name=f"natv{p}")
                        nat = nats[p]
                        ps = mmps.tile([128, 512], F32, tag="mm",
                                       name="projps")
                        for kc in range(8):
                            nc.tensor.matmul(
                                ps,
                                lhsT=xb_sb[:, kc, p, :],
                                rhs=wv_sb[:, kc, jh * 512:(jh + 1) * 512],
                                start=(kc == 0), stop=(kc == 7),
                            )
                        nc.vector.tensor_copy(
                            nat[:, jh * 512:(jh + 1) * 512], ps)
                        if jh == 1:
                            shr = vsh[p].rearrange(
                                "(il pp2) (t j) -> t il pp2 j",
                                il=8, t=16)[:, :, :, 0:64]
                            nc.gpsimd.dma_start(out=shr, in_=nat[:])
                            nc.scalar.dma_start(out=vh[p][:], in_=vsh[p])
                            nc.gpsimd.memset(
                                vh[p].rearrange(
                                    "q (b c) -> q b c", c=65)[:, :, 64],
                                1.0)  # ones column at 65b+64
                    return emit
                return [mk(p, jh) for p in pairs for jh in range(2)]

            def emit_attention_fused(ptp, rp, o2p, ypool, wo_sb):
                """Both groups' attention as ONE software-pipelined stream:
                the PV matmuls for unit n are emitted AFTER the S matmuls of
                unit n+1 (psum stA/stB rotate at depth 2), and the pipeline
                crosses the group boundary so group 0's drain overlaps group
                1's ramp-up.  Group 0's output projection is injected a few
                units after its last normalization."""
                o2s = {g: [o2p.tile([128, 8 * 128], BF16, tag=f"o2_{i}",
                                    name=f"o2_{g}_{i}") for i in range(2)]
                       for g in range(NG)}
                pvs_by = {}

                def emit_pv(g, a, gg, i, pt):
                    diag = gg >= 2 * a
                    d = gg - 2 * a
                    for q2 in range(2):
                        bb = 2 * gg + q2
                        if not diag:
                            nc.tensor.matmul(
                                pvs_by[(g, a)][i],
                                lhsT=vh[2 * g + i][:, bb * 65:bb * 65 + 65],
                                rhs=pt[:, q2 * 512:(q2 + 1) * 512],
                                start=(bb == 0),
                                stop=(bb == 4 * a + 3),
                            )
                        else:
                            m = 2 * d + q2
                            lo = q2 * 512 + 128 * m
                            nc.tensor.matmul(
                                pvs_by[(g, a)][i][:, 128 * m:512],
                                lhsT=vh[2 * g + i][:, bb * 65:bb * 65 + 65],
                                rhs=pt[:, lo:(q2 + 1) * 512],
                                start=(bb == 0),
                                stop=(bb == 4 * a + 3),
                                skip_group_check=True,
                            )

                def emit_norm(g, a):
                    pvs = pvs_by.pop((g, a))
                    for i in range(2):
                        r1 = rp.tile([1, 512], F32, tag="r1", name="r1_t")
                        nc.vector.reciprocal(r1, pvs[i][64:65, :])
                        rb = rp.tile([64, 512], F32, tag="rb", name="rb_t")
                        nc.gpsimd.partition_broadcast(rb, r1)
                        # o2[64*par + j2, 128u + 32a + r'] =
                        #     pvs[j2, 16r' + 2u + par] * rb[...]
                        pv_v = pvs[i][0:64, :].rearrange(
                            "j (rr uu two) -> j two uu rr", two=2, uu=8)
                        rb_v = rb.rearrange(
                            "j (rr uu two) -> j two uu rr", two=2, uu=8)
                        o2_v = o2s[g][i].rearrange(
                            "q (u rr) -> q u rr", u=8)[:, :, 32 * a:32 * a + 32]
                        for par in range(2):
                            nc.vector.tensor_mul(
                                o2_v[64 * par:64 * par + 64],
                                pv_v[:, par], rb_v[:, par])

                pending = None
                inter = []
                sched = []
                rot = [0]

                def next_tag():
                    rot[0] += 1
                    return "stA" if rot[0] % 2 == 0 else "stB"

                units = [(g, a, gg, i) for g in range(NG) for a in range(4)
                         for gg in range(2 * a + 2) for i in range(2)]
                for ui, (g, a, gg, i) in enumerate(units):
                    if inter and sched and ui >= sched[0]:
                        sched.pop(0)
                        inter.pop(0)(next_tag())
                    if gg == 0 and i == 0:
                        pvs_by[(g, a)] = [
                            mmps.tile([65, 512], F32, tag="mm",
                                      name=f"pv{g}_{a}_{ii}")
                            for ii in range(2)]
                    diag = gg >= 2 * a
                    d = gg - 2 * a
                    sts = stps.tile([128, 1024], F32, tag=next_tag(),
                                    name=f"st{g}_{a}_{gg}_{i}")
                    for q2 in range(2):
                        bb = 2 * gg + q2
                        # diagonal blocks m=1,2 only need cols >= 128m (m=3
                        # would drop the free dim under 256 for no gain)
                        m = 2 * d + q2 if diag else 0
                        off = 128 * m if m in (1, 2) else 0
                        nc.tensor.matmul(
                            sts[:, q2 * 512 + off:(q2 + 1) * 512],
                            lhsT=kht2[g][64 * i:64 * i + 64,
                                         bb * 128:(bb + 1) * 128],
                            rhs=qht2[g][64 * i:64 * i + 64,
                                        a * 512 + off:(a + 1) * 512],
                            start=True, stop=True,
                        )
                    pt = ptp.tile([128, 1024], BF16, tag="pt",
                                  name=f"pt_{g}_{a}_{gg}_{i}")
                    if not diag:
                        nc.scalar.activation(pt, sts, EXP)
                    else:
                        # exp only the valid columns; zero the boundary
                        # strip's upper triangle with the 0/1 mask
                        for q2 in range(2):
                            m = 2 * d + q2
                            lo = q2 * 512 + 128 * m
                            hi = (q2 + 1) * 512
                            nc.scalar.activation(
                                pt[:, lo:hi], sts[:, lo:hi], EXP)
                            nc.vector.tensor_mul(
                                pt[:, lo:lo + 128],
                                pt[:, lo:lo + 128], tri01)
                    if pending is not None:
                        emit_pv(*pending)
                        pg, pa, pgg, pi, _ = pending
                        if pgg == 2 * pa + 1 and pi == 1:
                            emit_norm(pg, pa)
                            if pg == 0 and pa == 3:
                                inter = emit_y_parts(0, o2s[0], wo_sb, ypool)
                                sched = [ui + 6, ui + 10]
                    pending = (g, a, gg, i, pt)
                emit_pv(*pending)
                emit_norm(pending[0], pending[1])
                return o2s

            def emit_y_parts(g, o2, wo_sb, ypool):
                """Returns closures [part_jh0, part_jh1]; each emits half of
                the output projection so it can interleave with the next
                group's attention stream."""
                ysbs = [ypool.tile([128, 1024], F32, tag="ysb",
                                   name=f"ysb{g}_{i}") for i in range(2)]

                def part(i, tag=None):
                    for jh in range(2):
                        yps = mmps.tile([128, 512], F32, tag="mm",
                                        name=f"ypsum_{i}_{jh}")
                        for u in range(8):
                            nc.tensor.matmul(
                                yps,
                                lhsT=o2[i][:, u * 128:(u + 1) * 128],
                                rhs=wo_sb[:, u * 1024 + jh * 512:
                                          u * 1024 + (jh + 1) * 512],
                                start=(u == 0), stop=(u == 7),
                            )
                        nc.vector.tensor_add(
                            ysbs[i][:, jh * 512:(jh + 1) * 512], yps,
                            bias_sb[:, jh * 512:(jh + 1) * 512])
                        nc.sync.dma_start(
                            out=out[2 * g + i][:, jh * 512:(jh + 1) * 512],
                            in_=ysbs[i][:, jh * 512:(jh + 1) * 512])

                return [lambda tag=None: part(0), lambda tag=None: part(1)]

            with (
                tc.tile_pool(name="xtp", bufs=1) as xtp,
                tc.tile_pool(name="wp", bufs=1) as wp,
                tc.tile_pool(name="pwork", bufs=1) as pwork,
                tc.tile_pool(name="p2", bufs=1) as p2,
                tc.tile_pool(name="ptp", bufs=6) as ptp,
                tc.tile_pool(name="rp", bufs=2) as rp,
                tc.tile_pool(name="o2p", bufs=2) as o2p,
                tc.tile_pool(name="yp", bufs=2) as ypool,
            ):
                xq_sb = xtp.tile([128, 8, PPC, 128], F32R, tag="xq",
                                 name="xqsb")
                xv = xt.rearrange("p (kc pr r) -> p kc pr r", kc=8, pr=PPC)
                for kc in range(8):
                    nc.scalar.dma_start(out=xq_sb[:, kc], in_=xv[:, kc])
                xb_sb = xtp.tile([128, 8, PPC, 128], BF16, tag="xb",
                                 name="xbsb")
                nc.scalar.dma_start(out=xb_sb, in_=xtb.rearrange(
                    "p (kc pr r) -> p kc pr r", kc=8, pr=PPC))

                w_sbs = []
                for wi, (wparam, dt_) in enumerate(
                        ((wv, BF16), (wq, F32R), (wk, F32R))):
                    w_sb = wp.tile([128, 8, 1024], dt_, tag=f"w{wi}",
                                   name=f"w{wi}")
                    for kc in range(8):
                        nc.sync.dma_start(
                            out=w_sb[:, kc, :],
                            in_=wparam.rearrange(
                                "(c p) j -> p c j", p=128)[:, kc, :])
                    w_sbs.append(w_sb)
                wo_sb = p2.tile([128, 8 * 1024], BF16, tag="wo")
                nc.sync.dma_start(out=wo_sb, in_=wo[:])

                # group 0 runs as early as possible; group 1's projections
                # and pairs 2/3's V path are injected into attention(g0)'s
                # ACT-bound stream at psum-quiet unit indices
                pre_rot = [0]

                def pre_tag():
                    pre_rot[0] += 1
                    return "stA" if pre_rot[0] % 2 == 0 else "stB"

                q0 = qk_chunks(w_sbs[1], xq_sb, qht2, 0, "q")
                q1 = qk_chunks(w_sbs[1], xq_sb, qht2, 1, "q")
                k0 = qk_chunks(w_sbs[2], xq_sb, kht2, 0, "k")
                k1 = qk_chunks(w_sbs[2], xq_sb, kht2, 1, "k")
                for ch in v_chunks(w_sbs[0], xb_sb, pwork, [0, 1]):
                    ch()
                for ch in (q0[0], q1[0], q0[1], q1[1]):
                    ch(pre_tag())
                for ch in (k0[0], k1[0], k0[1], k1[1]):
                    ch(pre_tag())
                for ch in v_chunks(w_sbs[0], xb_sb, pwork, [2, 3]):
                    ch()
                nc.scalar.dma_start(out=bias_sb, in_=bias[:])

                o2s = emit_attention_fused(ptp, rp, o2p, ypool, wo_sb)
                for part in emit_y_parts(1, o2s[1], wo_sb, ypool):
                    part()

    nc.finalize()
    return nc


def _host_prep(input_seq_embs, W_Q, W_K, W_V, W_O, b_O):
    X = np.asarray(input_seq_embs, dtype=np.float32)
    WQ = np.asarray(W_Q, dtype=np.float32)
    WK = np.asarray(W_K, dtype=np.float32)
    WV = np.asarray(W_V, dtype=np.float32)
    WO = np.asarray(W_O, dtype=np.float32)
    bO = np.asarray(b_O, dtype=np.float32)

    import ml_dtypes
    bf16 = ml_dtypes.bfloat16

    wq_arr = np.ascontiguousarray(WQ.T)
    wk_arr = np.ascontiguousarray(WK.T)
    wv_arr = np.ascontiguousarray(WV.T).astype(bf16)
    # wo[64*parity + j2, 1024*u + jo] = W_O.T[64*(2u+parity) + j2, jo]
    wo_arr = np.ascontiguousarray(
        WO.T.reshape(8, 2, 64, 1024).transpose(1, 2, 0, 3).reshape(
            128, 8192)).astype(bf16)
    bias_arr = np.ascontiguousarray(
        np.broadcast_to(bO, (128, 1024)).astype(np.float32))

    in_maps = []
    for c in range(NCORES):
        # xt[p_, kc, pair, r] = X_pair[r, 128*kc + p_]
        xts = np.empty((128, 8, PPC, 128), dtype=np.float32)
        for p in range(PPC):
            g = PPC * c + p
            bb, hh = g // H, g % H
            Xs = X[bb, 128 * hh:128 * (hh + 1), :]      # (128 r, 1024 cin)
            xts[:, :, p, :] = Xs.T.reshape(8, 128, 128).transpose(1, 0, 2)
        xt_arr = np.ascontiguousarray(xts.reshape(128, 8 * PPC * 128))
        in_maps.append({
            "xt": xt_arr, "xtb": xt_arr.astype(bf16),
            "wq": wq_arr, "wk": wk_arr, "wv": wv_arr, "wo": wo_arr,
            "bias": bias_arr,
        })
    return in_maps


_CACHED_NC = None


def get_nc():
    global _CACHED_NC
    if _CACHED_NC is None:
        _CACHED_NC = build_nc()
    return _CACHED_NC


def kernel(**inputs) -> np.ndarray:
    nc = get_nc()
    in_maps = _host_prep(**inputs)
    res = run_bass_kernel_spmd(nc, in_maps, list(range(NCORES)))
    out = np.empty((B, L, D), dtype=np.float32)
    for c in range(NCORES):
        y = res.results[c]["out"]  # (4, 128, 1024)
        for p in range(PPC):
            g = PPC * c + p
            bb, hh = g // H, g % H
            out[bb, 128 * hh:128 * (hh + 1), :] = y[p]
    return out


# revision 6
# speedup vs baseline: 7.7176x; 1.0158x over previous
"""Trainium2 Bass kernel for nn_MultiHeadAttention_66322884984909.

Math (faithful to reference):
  Q = X @ W_Q.T reshaped (B, H, L, hd) via DIRECT reshape -> head h owns rows
  128h:128(h+1) of the projected (L, D) matrix, reinterpreted as (L=2048, hd=64).
  Heads are sequence-parallel: 32 (batch, head) pairs, 8 cores x 4 pairs.

Design:
  - Q/K computed PRE-TRANSPOSED: Qf^T = W_Q.T' @ X_s^T with output partitions
    = W-output-columns, batching a group's 2 pairs in the matmul free dim.
    psum quarters [(t-parity, j), (pair, r)] land in qht2/kht2[64i+j,
    s=16r+2c+parity] via strided DVE copies -- no DRAM shuffle bounce and no
    PE transposes for Q/K.  (psum start/stop act on whole 2KB banks: only the
    first/last quarter of a bank starts/stops its accumulation group.)
  - Causal masking without an inner-loop affine_select: on diagonal S tiles,
    exp and P@V read only columns q >= 128m of each key block; the 128-wide
    boundary strip is zeroed by one bf16 DVE multiply with a constant 0/1
    triangle.  The m=0 block is full width, so the psum accumulation group
    start stays uniform (bb == 0).
  - Attention is software-pipelined per unit (a-chunk, key-block-pair, pair):
    S(n+1) is emitted BEFORE PV(n) so the PE never idles on the exp latency;
    sts psum tiles double-buffer via a shared stA/stB tag rotation.
    Softmax row sums ride along as a 65th ones-column of V; normalization
    (reciprocal + partition_broadcast + strided muls) scatters O directly
    into the o2 layout [(s-parity, j2), (u, r)].
  - Output projection contracts 128-deep (8 accumulating matmuls per 512-col
    half), split into per-pair closures interleaved into group 1's
    attention stream.  Both groups' attention form ONE fused pipeline, so
    group 0's drain overlaps group 1's ramp-up.  W_V loads first and the
    descriptor-heavy V shuffle-scatter overlaps the W_Q/W_K streams;
    dummy matmuls/exp ramp the PE p-state and activation table during the
    initial DMA wait (kc is the outer loop so matmuls chase the W DMAs).
  - bf16 on the linear-only paths (W_V + X-for-V, the V DRAM bounce, exp
    output P, O, W_O): halves their DMA/SBUF cost with ~3e-3 relative error
    (the exp-amplified Q/K path stays fp32r).
  - No max-subtraction in softmax: logits ~ N(0, 64); exp stays finite in
    fp32.  All fp32 matmuls run as fp32r with free dim >= 256 (full rate).

Cost-model (TimelineSim) total: 168.1 us vs 248.9 us for the v1 baseline.
"""

import numpy as np

import concourse.bass as bass
from concourse import bacc
import concourse.mybir as mybir
import concourse.tile as tile
from concourse.bass_utils import run_bass_kernel_spmd

F32 = mybir.dt.float32
F32R = mybir.dt.float32r
BF16 = mybir.dt.bfloat16
EXP = mybir.ActivationFunctionType.Exp

B, L, D = 2, 2048, 1024
H, HD = 16, 64
NCORES = 8
PPC = 4   # pairs per core
NG = 2    # groups of 2 pairs
NEG = -1.0e30


def build_nc(repeat=1):
    nc = bacc.Bacc(trn_type="TRN2", target_bir_lowering=False, debug=False)

    # xt[p_, kc*512 + 128*pair + r] = X_pair[r, 128*kc + p_]
    xt = nc.declare_dram_parameter("xt", [128, 8 * PPC * 128], F32R,
                                   isOutput=False)
    # xtb: same layout as xt, bf16 (feeds the V projection)
    xtb = nc.declare_dram_parameter("xtb", [128, 8 * PPC * 128], BF16,
                                    isOutput=False)
    wq = nc.declare_dram_parameter("wq", [1024, 1024], F32R, isOutput=False)
    wk = nc.declare_dram_parameter("wk", [1024, 1024], F32R, isOutput=False)
    wv = nc.declare_dram_parameter("wv", [1024, 1024], BF16, isOutput=False)
    # wo[64*parity + j2, 1024*u + jo] = W_O.T[64*(2u+parity) + j2, jo]
    wo = nc.declare_dram_parameter("wo", [128, 8 * 1024], BF16, isOutput=False)
    bias = nc.declare_dram_parameter("bias", [128, 1024], F32, isOutput=False)
    out = nc.declare_dram_parameter("out", [PPC, 128, 1024], F32, isOutput=True)
    vsh = nc.dram_tensor("vsh", [PPC, 128, 1040], BF16)

    with tile.TileContext(nc) as tc:
      for _rep in range(repeat):
        with (
            tc.tile_pool(name="consts", bufs=1) as consts,
            tc.tile_pool(name="headt", bufs=1) as headt,
            tc.tile_pool(name="mmps", bufs=4, space="PSUM") as mmps,
            tc.tile_pool(name="stps", bufs=1, space="PSUM") as stps,
        ):
            bias_sb = consts.tile([128, 1024], F32)
            # tri01[k, q'] = 1.0 if q' >= k else 0.0 (strip causal mask)
            tri01 = consts.tile([128, 128], BF16)
            nc.gpsimd.memset(tri01, 1.0)
            # dummy exp to preload the activation table during the
            # projection phase (the lazy load costs 1.3us otherwise)
            warm = consts.tile([128, 1], F32, tag="warm")
            nc.gpsimd.memset(warm, 0.0)
            nc.scalar.activation(warm, warm, EXP)
            # dummy matmuls ramp the PE p-state (full clock needs ~3us of
            # continuous execution) while the first weight DMAs stream in
            wmm = consts.tile([128, 256], F32, tag="wmm")
            nc.gpsimd.memset(wmm, 0.0)
            for _w in range(6):
                pw = mmps.tile([128, 256], F32, tag="mm", name=f"warmmm{_w}")
                nc.tensor.matmul(pw, lhsT=wmm[:, 0:128], rhs=wmm,
                                 start=True, stop=True)
            nc.gpsimd.affine_select(
                out=tri01, in_=tri01,
                compare_op=mybir.AluOpType.is_ge,
                fill=0.0,
                base=0,
                pattern=[[1, 128]],
                channel_multiplier=-1,
            )

            # qht2/kht2[g]: [64*i + j, s] for pair 2g+i  (j = head dim)
            qht2 = [headt.tile([128, 2048], F32R, tag=f"qht{g}", name=f"qht{g}")
                    for g in range(NG)]
            kht2 = [headt.tile([128, 2048], F32R, tag=f"kht{g}", name=f"kht{g}")
                    for g in range(NG)]
            # vh[p]: [s-in-block, 65*bb + j] with ones column at j=64
            vh = [headt.tile([128, 16 * 65], BF16, tag=f"vh{p}", name=f"vh{p}")
                  for p in range(PPC)]

            def qk_chunks(w_sb, xq_sb, dsts, g, wname):
                """Transposed projection, as 4 chunk-closures per (W, g).

                Chunk (half, bank): one psum tile [128, 512] = 2 c-quarters
                (c = 4*half + 2*bank + cq).  Quarter c holds [(parity, j),
                (i, r)] with W-output column 128c + 64*parity + j; copies land
                in dsts[g][64i + j, s = 16r + 2c + parity].  psum start/stop
                act on whole 2KB banks, so only the first quarter starts and
                only the last stops the accumulation group.
                """
                def mk(half):
                    def emit(tag="stA"):
                        pg = stps.tile(
                            [128, 1024], F32, tag=tag,
                            name=f"prj_{wname}_{g}_{half}")
                        for kc in range(8):
                            for c4 in range(4):
                                c = 4 * half + c4
                                # psum start/stop act on whole 2KB banks: a
                                # bank spans two 256-col quarters, so only the
                                # first quarter starts / last quarter stops
                                nc.tensor.matmul(
                                    pg[:, 256 * c4:256 * c4 + 256],
                                    lhsT=w_sb[:, kc, c * 128:(c + 1) * 128],
                                    rhs=xq_sb[:, kc, 2 * g:2 * g + 2, :],
                                    start=(kc == 0 and c4 % 2 == 0),
                                    stop=(kc == 7 and c4 % 2 == 1),
                                )
                        v = dsts[g].rearrange(
                            "(i j) (r c two) -> i j two c r", i=2, c=8, two=2)
                        pv4 = pg.rearrange(
                            "p (c4 two r) -> p c4 two r", c4=4, two=2)
                        for parity in range(2):
                            for i in range(2):
                                nc.vector.tensor_copy(
                                    v[i, :, parity, 4 * half:4 * half + 4, :],
                                    pv4[64 * parity:64 * parity + 64, :, i, :])
                    return emit
                return [mk(h) for h in range(2)]

            def v_chunks(wv_sb, xb_sb, pwork, pairs):
                """V projection + DRAM shuffle bounce, one closure per
                (pair, jh-half)."""
                nats = {}

                def mk(p, jh):
                    def emit(tag=None):
                        if jh == 0:
                            nats[p] = pwork.tile([128, 1024], BF16, tag="natv",
                                                 bufs=2, name=f"natv{p}")
                        nat = nats[p]
                        ps = mmps.tile([128, 512], F32, tag="mm",
                                       name="projps")
                        for kc in range(8):
                            nc.tensor.matmul(
                                ps,
                                lhsT=xb_sb[:, kc, p, :],
                                rhs=wv_sb[:, kc, jh * 512:(jh + 1) * 512],
                                start=(kc == 0), stop=(kc == 7),
                            )
                        nc.vector.tensor_copy(
                            nat[:, jh * 512:(jh + 1) * 512], ps)
                        if jh == 1:
                            shr = vsh[p].rearrange(
                                "(il pp2) (t j) -> t il pp2 j",
                                il=8, t=16)[:, :, :, 0:64]
                            nc.gpsimd.dma_start(out=shr, in_=nat[:])
                            nc.scalar.dma_start(out=vh[p][:], in_=vsh[p])
                            nc.gpsimd.memset(
                                vh[p].rearrange(
                                    "q (b c) -> q b c", c=65)[:, :, 64],
                                1.0)  # ones column at 65b+64
                    return emit
                return [mk(p, jh) for p in pairs for jh in range(2)]

            def emit_attention_fused(ptp, rp, o2p, ypool, wo_sb):
                """Both groups' attention as ONE software-pipelined stream:
                the PV matmuls for unit n are emitted AFTER the S matmuls of
                unit n+1 (psum stA/stB rotate at depth 2), and the pipeline
                crosses the group boundary so group 0's drain overlaps group
                1's ramp-up.  Group 0's output projection is injected a few
                units after its last normalization."""
                o2s = {g: [o2p.tile([128, 8 * 128], BF16, tag=f"o2_{i}",
                                    name=f"o2_{g}_{i}") for i in range(2)]
                       for g in range(NG)}
                pvs_by = {}

                def emit_pv(g, a, gg, i, pt):
                    diag = gg >= 2 * a
                    d = gg - 2 * a
                    for q2 in range(2):
                        bb = 2 * gg + q2
                        if not diag:
                            nc.tensor.matmul(
                                pvs_by[(g, a)][i],
                                lhsT=vh[2 * g + i][:, bb * 65:bb * 65 + 65],
                                rhs=pt[:, q2 * 512:(q2 + 1) * 512],
                                start=(bb == 0),
                                stop=(bb == 4 * a + 3),
                            )
                        else:
                            m = 2 * d + q2
                            lo = q2 * 512 + 128 * m
                            nc.tensor.matmul(
                                pvs_by[(g, a)][i][:, 128 * m:512],
                                lhsT=vh[2 * g + i][:, bb * 65:bb * 65 + 65],
                                rhs=pt[:, lo:(q2 + 1) * 512],
                                start=(bb == 0),
                                stop=(bb == 4 * a + 3),
                                skip_group_check=True,
                            )

                def emit_norm_one(g, a, i):
                    pvs = pvs_by[(g, a)]
                    r1 = rp.tile([1, 512], F32, tag="r1", name="r1_t")
                    nc.vector.reciprocal(r1, pvs[i][64:65, :])
                    rb = rp.tile([64, 512], F32, tag="rb", name="rb_t")
                    nc.gpsimd.partition_broadcast(rb, r1)
                    # o2[64*par + j2, 128u + 32a + r'] =
                    #     pvs[j2, 16r' + 2u + par] * rb[...]
                    pv_v = pvs[i][0:64, :].rearrange(
                        "j (rr uu two) -> j two uu rr", two=2, uu=8)
                    rb_v = rb.rearrange(
                        "j (rr uu two) -> j two uu rr", two=2, uu=8)
                    o2_v = o2s[g][i].rearrange(
                        "q (u rr) -> q u rr", u=8)[:, :, 32 * a:32 * a + 32]
                    for par in range(2):
                        nc.vector.tensor_mul(
                            o2_v[64 * par:64 * par + 64],
                            pv_v[:, par], rb_v[:, par])
                    if i == 1:
                        pvs_by.pop((g, a))

                pending = None
                inter = []
                sched = []
                tail_parts = None
                rot = [0]

                def next_tag():
                    rot[0] += 1
                    return "stA" if rot[0] % 2 == 0 else "stB"

                units = [(g, a, gg, i) for g in range(NG) for a in range(4)
                         for gg in range(2 * a + 2) for i in range(2)]
                for ui, (g, a, gg, i) in enumerate(units):
                    if inter and sched and ui >= sched[0]:
                        sched.pop(0)
                        inter.pop(0)(next_tag())
                    if gg == 0 and i == 0:
                        pvs_by[(g, a)] = [
                            mmps.tile([65, 512], F32, tag="mm",
                                      name=f"pv{g}_{a}_{ii}")
                            for ii in range(2)]
                    diag = gg >= 2 * a
                    d = gg - 2 * a
                    sts = stps.tile([128, 1024], F32, tag=next_tag(),
                                    name=f"st{g}_{a}_{gg}_{i}")
                    for q2 in range(2):
                        bb = 2 * gg + q2
                        # diagonal blocks m=1,2 only need cols >= 128m (m=3
                        # would drop the free dim under 256 for no gain)
                        m = 2 * d + q2 if diag else 0
                        off = 128 * m if m in (1, 2) else 0
                        nc.tensor.matmul(
                            sts[:, q2 * 512 + off:(q2 + 1) * 512],
                            lhsT=kht2[g][64 * i:64 * i + 64,
                                         bb * 128:(bb + 1) * 128],
                            rhs=qht2[g][64 * i:64 * i + 64,
                                        a * 512 + off:(a + 1) * 512],
                            start=True, stop=True,
                        )
                    pt = ptp.tile([128, 1024], BF16, tag="pt",
                                  name=f"pt_{g}_{a}_{gg}_{i}")
                    if not diag:
                        nc.scalar.activation(pt, sts, EXP)
                    elif d == 0:
                        # one full-width exp (cols [512:640) are garbage the
                        # restricted PV never reads); strips masked after
                        nc.scalar.activation(pt, sts, EXP)
                        nc.vector.tensor_mul(
                            pt[:, 0:128], pt[:, 0:128], tri01)
                        nc.vector.tensor_mul(
                            pt[:, 640:768], pt[:, 640:768], tri01)
                    else:
                        # exp only the valid columns; zero the boundary
                        # strip's upper triangle with the 0/1 mask
                        for q2 in range(2):
                            m = 2 * d + q2
                            lo = q2 * 512 + 128 * m
                            hi = (q2 + 1) * 512
                            nc.scalar.activation(
                                pt[:, lo:hi], sts[:, lo:hi], EXP)
                            nc.vector.tensor_mul(
                                pt[:, lo:lo + 128],
                                pt[:, lo:lo + 128], tri01)
                    if pending is not None:
                        emit_pv(*pending)
                        pg, pa, pgg, pi, _ = pending
                        if pgg == 2 * pa + 1:
                            # this pair's PV chain just completed
                            emit_norm_one(pg, pa, pi)
                            if pg == 0 and pa == 3 and pi == 1:
                                inter = emit_y_parts(0, o2s[0], wo_sb, ypool)
                                sched = [ui + 6, ui + 10]
                            if pg == 1 and pa == 3 and pi == 0:
                                # overlap g1's first output-projection half
                                # with the final unit's PV + normalization
                                tail_parts = emit_y_parts(1, o2s[1], wo_sb,
                                                          ypool)
                                tail_parts[0]()
                    pending = (g, a, gg, i, pt)
                emit_pv(*pending)
                emit_norm_one(pending[0], pending[1], pending[3])
                tail_parts[1]()
                return o2s

            def emit_y_parts(g, o2, wo_sb, ypool):
                """Returns closures [part_jh0, part_jh1]; each emits half of
                the output projection so it can interleave with the next
                group's attention stream."""
                ysbs = [ypool.tile([128, 1024], F32, tag="ysb",
                                   name=f"ysb{g}_{i}") for i in range(2)]

                def part(i, tag=None):
                    for jh in range(2):
                        yps = mmps.tile([128, 512], F32, tag="mm",
                                        name=f"ypsum_{i}_{jh}")
                        for u in range(8):
                            nc.tensor.matmul(
                                yps,
                                lhsT=o2[i][:, u * 128:(u + 1) * 128],
                                rhs=wo_sb[:, u * 1024 + jh * 512:
                                          u * 1024 + (jh + 1) * 512],
                                start=(u == 0), stop=(u == 7),
                            )
                        nc.vector.tensor_add(
                            ysbs[i][:, jh * 512:(jh + 1) * 512], yps,
                            bias_sb[:, jh * 512:(jh + 1) * 512])
                        nc.sync.dma_start(
                            out=out[2 * g + i][:, jh * 512:(jh + 1) * 512],
                            in_=ysbs[i][:, jh * 512:(jh + 1) * 512])

                return [lambda tag=None: part(0), lambda tag=None: part(1)]

            with (
                tc.tile_pool(name="xtp", bufs=1) as xtp,
                tc.tile_pool(name="wp", bufs=1) as wp,
                tc.tile_pool(name="pwork", bufs=1) as pwork,
                tc.tile_pool(name="p2", bufs=1) as p2,
                tc.tile_pool(name="ptp", bufs=6) as ptp,
                tc.tile_pool(name="rp", bufs=2) as rp,
                tc.tile_pool(name="o2p", bufs=2) as o2p,
                tc.tile_pool(name="yp", bufs=2) as ypool,
            ):
                xq_sb = xtp.tile([128, 8, PPC, 128], F32R, tag="xq",
                                 name="xqsb")
                xv = xt.rearrange("p (kc pr r) -> p kc pr r", kc=8, pr=PPC)
                for kc in range(8):
                    nc.scalar.dma_start(out=xq_sb[:, kc], in_=xv[:, kc])
                xb_sb = xtp.tile([128, 8, PPC, 128], BF16, tag="xb",
                                 name="xbsb")
                nc.scalar.dma_start(out=xb_sb, in_=xtb.rearrange(
                    "p (kc pr r) -> p kc pr r", kc=8, pr=PPC))

                w_sbs = []
                for wi, (wparam, dt_) in enumerate(
                        ((wv, BF16), (wq, F32R), (wk, F32R))):
                    w_sb = wp.tile([128, 8, 1024], dt_, tag=f"w{wi}",
                                   name=f"w{wi}")
                    for kc in range(8):
                        nc.sync.dma_start(
                            out=w_sb[:, kc, :],
                            in_=wparam.rearrange(
                                "(c p) j -> p c j", p=128)[:, kc, :])
                    w_sbs.append(w_sb)
                wo_sb = p2.tile([128, 8 * 1024], BF16, tag="wo")
                nc.sync.dma_start(out=wo_sb, in_=wo[:])

                # group 0 runs as early as possible; group 1's projections
                # and pairs 2/3's V path are injected into attention(g0)'s
                # ACT-bound stream at psum-quiet unit indices
                pre_rot = [0]

                def pre_tag():
                    pre_rot[0] += 1
                    return "stA" if pre_rot[0] % 2 == 0 else "stB"

                q0 = qk_chunks(w_sbs[1], xq_sb, qht2, 0, "q")
                q1 = qk_chunks(w_sbs[1], xq_sb, qht2, 1, "q")
                k0 = qk_chunks(w_sbs[2], xq_sb, kht2, 0, "k")
                k1 = qk_chunks(w_sbs[2], xq_sb, kht2, 1, "k")
                for ch in v_chunks(w_sbs[0], xb_sb, pwork, [0, 1]):
                    ch()
                for ch in (q0[0], q1[0], q0[1], q1[1]):
                    ch(pre_tag())
                for ch in (k0[0], k1[0], k0[1], k1[1]):
                    ch(pre_tag())
                for ch in v_chunks(w_sbs[0], xb_sb, pwork, [2, 3]):
                    ch()
                nc.scalar.dma_start(out=bias_sb, in_=bias[:])

                emit_attention_fused(ptp, rp, o2p, ypool, wo_sb)

    nc.finalize()
    return nc


def _host_prep(input_seq_embs, W_Q, W_K, W_V, W_O, b_O):
    X = np.asarray(input_seq_embs, dtype=np.float32)
    WQ = np.asarray(W_Q, dtype=np.float32)
    WK = np.asarray(W_K, dtype=np.float32)
    WV = np.asarray(W_V, dtype=np.float32)
    WO = np.asarray(W_O, dtype=np.float32)
    bO = np.asarray(b_O, dtype=np.float32)

    import ml_dtypes
    bf16 = ml_dtypes.bfloat16

    wq_arr = np.ascontiguousarray(WQ.T)
    wk_arr = np.ascontiguousarray(WK.T)
    wv_arr = np.ascontiguousarray(WV.T).astype(bf16)
    # wo[64*parity + j2, 1024*u + jo] = W_O.T[64*(2u+parity) + j2, jo]
    wo_arr = np.ascontiguousarray(
        WO.T.reshape(8, 2, 64, 1024).transpose(1, 2, 0, 3).reshape(
            128, 8192)).astype(bf16)
    bias_arr = np.ascontiguousarray(
        np.broadcast_to(bO, (128, 1024)).astype(np.float32))

    in_maps = []
    for c in range(NCORES):
        # xt[p_, kc, pair, r] = X_pair[r, 128*kc + p_]
        xts = np.empty((128, 8, PPC, 128), dtype=np.float32)
        for p in range(PPC):
            g = PPC * c + p
            bb, hh = g // H, g % H
            Xs = X[bb, 128 * hh:128 * (hh + 1), :]      # (128 r, 1024 cin)
            xts[:, :, p, :] = Xs.T.reshape(8, 128, 128).transpose(1, 0, 2)
        xt_arr = np.ascontiguousarray(xts.reshape(128, 8 * PPC * 128))
        in_maps.append({
            "xt": xt_arr, "xtb": xt_arr.astype(bf16),
            "wq": wq_arr, "wk": wk_arr, "wv": wv_arr, "wo": wo_arr,
            "bias": bias_arr,
        })
    return in_maps


_CACHED_NC = None


def get_nc():
    global _CACHED_NC
    if _CACHED_NC is None:
        _CACHED_NC = build_nc()
    return _CACHED_NC


def kernel(**inputs) -> np.ndarray:
    nc = get_nc()
    in_maps = _host_prep(**inputs)
    res = run_bass_kernel_spmd(nc, in_maps, list(range(NCORES)))
    out = np.empty((B, L, D), dtype=np.float32)
    for c in range(NCORES):
        y = res.results[c]["out"]  # (4, 128, 1024)
        for p in range(PPC):
            g = PPC * c + p
            bb, hh = g // H, g % H
            out[bb, 128 * hh:128 * (hh + 1), :] = y[p]
    return out


# revision 7
# speedup vs baseline: 8.1676x; 1.0583x over previous
"""Trainium2 Bass kernel for nn_MultiHeadAttention_66322884984909.

Math (faithful to reference):
  Q = X @ W_Q.T reshaped (B, H, L, hd) via DIRECT reshape -> head h owns rows
  128h:128(h+1) of the projected (L, D) matrix, reinterpreted as (L=2048, hd=64).
  Heads are sequence-parallel: 32 (batch, head) pairs, 8 cores x 4 pairs.

Design:
  - Q/K computed PRE-TRANSPOSED: Qf^T = W_Q.T' @ X_s^T with output partitions
    = W-output-columns, batching a group's 2 pairs in the matmul free dim.
    psum quarters [(t-parity, j), (pair, r)] land in qht2/kht2[64i+j,
    s=16r+2c+parity] via strided DVE copies -- no DRAM shuffle bounce and no
    PE transposes for Q/K.  (psum start/stop act on whole 2KB banks: only the
    first/last quarter of a bank starts/stops its accumulation group.)
  - Causal masking without an inner-loop affine_select: on diagonal S tiles,
    exp and P@V read only columns q >= 128m of each key block; the 128-wide
    boundary strip is zeroed by one bf16 DVE multiply with a constant 0/1
    triangle.  The m=0 block is full width, so the psum accumulation group
    start stays uniform (bb == 0).
  - Attention is software-pipelined per unit (a-chunk, key-block-pair, pair):
    S(n+1) is emitted BEFORE PV(n) so the PE never idles on the exp latency;
    sts psum tiles double-buffer via a shared stA/stB tag rotation.
    Softmax row sums ride along as a 65th ones-column of V; normalization
    (reciprocal + partition_broadcast + strided muls) scatters O directly
    into the o2 layout [(s-parity, j2), (u, r)].
  - Output projection contracts 128-deep (8 accumulating matmuls per 512-col
    half), split into per-pair closures interleaved into group 1's
    attention stream.  Both groups' attention form ONE fused pipeline, so
    group 0's drain overlaps group 1's ramp-up.  W_V loads first and the
    descriptor-heavy V shuffle-scatter overlaps the W_Q/W_K streams;
    dummy matmuls/exp ramp the PE p-state and activation table during the
    initial DMA wait (kc is the outer loop so matmuls chase the W DMAs).
  - bf16 inputs and weights throughout (X, W_Q, W_K, W_V, W_O, the V DRAM
    bounce, exp output P, O): measured 1.05e-2 relative error against the
    2e-2 gate (softmax renormalization cancels most of the bf16 logit
    error); S and the psum accumulations stay fp32.  Halves the weight
    DMA lead-in that gates the attention start.
  - No max-subtraction in softmax: logits ~ N(0, 64); exp stays finite in
    fp32.  All fp32 matmuls run as fp32r with free dim >= 256 (full rate).

Cost-model (TimelineSim) total: 158.8 us vs 248.9 us for the v1 baseline.
"""

import numpy as np

import concourse.bass as bass
from concourse import bacc
import concourse.mybir as mybir
import concourse.tile as tile
from concourse.bass_utils import run_bass_kernel_spmd

F32 = mybir.dt.float32
F32R = mybir.dt.float32r
BF16 = mybir.dt.bfloat16
EXP = mybir.ActivationFunctionType.Exp

B, L, D = 2, 2048, 1024
H, HD = 16, 64
NCORES = 8
PPC = 4   # pairs per core
NG = 2    # groups of 2 pairs
NEG = -1.0e30


def build_nc(repeat=1):
    nc = bacc.Bacc(trn_type="TRN2", target_bir_lowering=False, debug=False)

    # xtb[p_, kc*512 + 128*pair + r] = X_pair[r, 128*kc + p_] (bf16)
    xtb = nc.declare_dram_parameter("xtb", [128, 8 * PPC * 128], BF16,
                                    isOutput=False)
    wq = nc.declare_dram_parameter("wq", [1024, 1024], BF16, isOutput=False)
    wk = nc.declare_dram_parameter("wk", [1024, 1024], BF16, isOutput=False)
    wv = nc.declare_dram_parameter("wv", [1024, 1024], BF16, isOutput=False)
    # wo[64*parity + j2, 1024*u + jo] = W_O.T[64*(2u+parity) + j2, jo]
    wo = nc.declare_dram_parameter("wo", [128, 8 * 1024], BF16, isOutput=False)
    bias = nc.declare_dram_parameter("bias", [128, 1024], F32, isOutput=False)
    out = nc.declare_dram_parameter("out", [PPC, 128, 1024], F32, isOutput=True)
    vsh = nc.dram_tensor("vsh", [PPC, 128, 1040], BF16)

    with tile.TileContext(nc) as tc:
      for _rep in range(repeat):
        with (
            tc.tile_pool(name="consts", bufs=1) as consts,
            tc.tile_pool(name="headt", bufs=1) as headt,
            tc.tile_pool(name="mmps", bufs=4, space="PSUM") as mmps,
            tc.tile_pool(name="stps", bufs=1, space="PSUM") as stps,
        ):
            bias_sb = consts.tile([128, 1024], F32)
            # tri01[k, q'] = 1.0 if q' >= k else 0.0 (strip causal mask)
            tri01 = consts.tile([128, 128], BF16)
            nc.gpsimd.memset(tri01, 1.0)
            # dummy exp to preload the activation table during the
            # projection phase (the lazy load costs 1.3us otherwise)
            warm = consts.tile([128, 1], F32, tag="warm")
            nc.gpsimd.memset(warm, 0.0)
            nc.scalar.activation(warm, warm, EXP)
            # dummy matmuls ramp the PE p-state (full clock needs ~3us of
            # continuous execution) while the first weight DMAs stream in
            wmm = consts.tile([128, 256], F32, tag="wmm")
            nc.gpsimd.memset(wmm, 0.0)
            for _w in range(6):
                pw = mmps.tile([128, 256], F32, tag="mm", name=f"warmmm{_w}")
                nc.tensor.matmul(pw, lhsT=wmm[:, 0:128], rhs=wmm,
                                 start=True, stop=True)
            nc.gpsimd.affine_select(
                out=tri01, in_=tri01,
                compare_op=mybir.AluOpType.is_ge,
                fill=0.0,
                base=0,
                pattern=[[1, 128]],
                channel_multiplier=-1,
            )

            # qht2/kht2[g]: [64*i + j, s] for pair 2g+i  (j = head dim)
            qht2 = [headt.tile([128, 2048], F32R, tag=f"qht{g}", name=f"qht{g}")
                    for g in range(NG)]
            kht2 = [headt.tile([128, 2048], F32R, tag=f"kht{g}", name=f"kht{g}")
                    for g in range(NG)]
            # vh[p]: [s-in-block, 65*bb + j] with ones column at j=64
            vh = [headt.tile([128, 16 * 65], BF16, tag=f"vh{p}", name=f"vh{p}")
                  for p in range(PPC)]

            def qk_chunks(w_sb, xq_sb, dsts, g, wname):
                """Transposed projection, as 4 chunk-closures per (W, g).

                Chunk (half, bank): one psum tile [128, 512] = 2 c-quarters
                (c = 4*half + 2*bank + cq).  Quarter c holds [(parity, j),
                (i, r)] with W-output column 128c + 64*parity + j; copies land
                in dsts[g][64i + j, s = 16r + 2c + parity].  psum start/stop
                act on whole 2KB banks, so only the first quarter starts and
                only the last stops the accumulation group.
                """
                def mk(half):
                    def emit(tag="stA"):
                        pg = stps.tile(
                            [128, 1024], F32, tag=tag,
                            name=f"prj_{wname}_{g}_{half}")
                        for kc in range(8):
                            for c4 in range(4):
                                c = 4 * half + c4
                                # psum start/stop act on whole 2KB banks: a
                                # bank spans two 256-col quarters, so only the
                                # first quarter starts / last quarter stops
                                nc.tensor.matmul(
                                    pg[:, 256 * c4:256 * c4 + 256],
                                    lhsT=w_sb[:, kc, c * 128:(c + 1) * 128],
                                    rhs=xq_sb[:, kc, 2 * g:2 * g + 2, :],
                                    start=(kc == 0 and c4 % 2 == 0),
                                    stop=(kc == 7 and c4 % 2 == 1),
                                )
                        v = dsts[g].rearrange(
                            "(i j) (r c two) -> i j two c r", i=2, c=8, two=2)
                        pv4 = pg.rearrange(
                            "p (c4 two r) -> p c4 two r", c4=4, two=2)
                        for parity in range(2):
                            for i in range(2):
                                nc.vector.tensor_copy(
                                    v[i, :, parity, 4 * half:4 * half + 4, :],
                                    pv4[64 * parity:64 * parity + 64, :, i, :])
                    return emit
                return [mk(h) for h in range(2)]

            def v_chunks(wv_sb, xb_sb, pwork, pairs):
                """V projection + DRAM shuffle bounce, one closure per
                (pair, jh-half)."""
                nats = {}

                def mk(p, jh):
                    def emit(tag=None):
                        if jh == 0:
                            nats[p] = pwork.tile([128, 1024], BF16, tag="natv",
                                                 bufs=2, name=f"natv{p}")
                        nat = nats[p]
                        ps = mmps.tile([128, 512], F32, tag="mm",
                                       name="projps")
                        for kc in range(8):
                            nc.tensor.matmul(
                                ps,
                                lhsT=xb_sb[:, kc, p, :],
                                rhs=wv_sb[:, kc, jh * 512:(jh + 1) * 512],
                                start=(kc == 0), stop=(kc == 7),
                            )
                        nc.vector.tensor_copy(
                            nat[:, jh * 512:(jh + 1) * 512], ps)
                        if jh == 1:
                            shr = vsh[p].rearrange(
                                "(il pp2) (t j) -> t il pp2 j",
                                il=8, t=16)[:, :, :, 0:64]
                            nc.gpsimd.dma_start(out=shr, in_=nat[:])
                            nc.scalar.dma_start(out=vh[p][:], in_=vsh[p])
                            nc.gpsimd.memset(
                                vh[p].rearrange(
                                    "q (b c) -> q b c", c=65)[:, :, 64],
                                1.0)  # ones column at 65b+64
                    return emit
                return [mk(p, jh) for p in pairs for jh in range(2)]

            def emit_attention_fused(ptp, rp, o2p, ypool, wo_sb):
                """Both groups' attention as ONE software-pipelined stream:
                the PV matmuls for unit n are emitted AFTER the S matmuls of
                unit n+1 (psum stA/stB rotate at depth 2), and the pipeline
                crosses the group boundary so group 0's drain overlaps group
                1's ramp-up.  Group 0's output projection is injected a few
                units after its last normalization."""
                o2s = {g: [o2p.tile([128, 8 * 128], BF16, tag=f"o2_{i}",
                                    name=f"o2_{g}_{i}") for i in range(2)]
                       for g in range(NG)}
                pvs_by = {}

                def emit_pv(g, a, gg, i, pt):
                    diag = gg >= 2 * a
                    d = gg - 2 * a
                    for q2 in range(2):
                        bb = 2 * gg + q2
                        if not diag:
                            nc.tensor.matmul(
                                pvs_by[(g, a)][i],
                                lhsT=vh[2 * g + i][:, bb * 65:bb * 65 + 65],
                                rhs=pt[:, q2 * 512:(q2 + 1) * 512],
                                start=(bb == 0),
                                stop=(bb == 4 * a + 3),
                            )
                        else:
                            m = 2 * d + q2
                            lo = q2 * 512 + 128 * m
                            nc.tensor.matmul(
                                pvs_by[(g, a)][i][:, 128 * m:512],
                                lhsT=vh[2 * g + i][:, bb * 65:bb * 65 + 65],
                                rhs=pt[:, lo:(q2 + 1) * 512],
                                start=(bb == 0),
                                stop=(bb == 4 * a + 3),
                                skip_group_check=True,
                            )

                def emit_norm_one(g, a, i):
                    pvs = pvs_by[(g, a)]
                    r1 = rp.tile([1, 512], F32, tag="r1", name="r1_t")
                    nc.vector.reciprocal(r1, pvs[i][64:65, :])
                    rb = rp.tile([64, 512], F32, tag="rb", name="rb_t")
                    nc.gpsimd.partition_broadcast(rb, r1)
                    # o2[64*par + j2, 128u + 32a + r'] =
                    #     pvs[j2, 16r' + 2u + par] * rb[...]
                    pv_v = pvs[i][0:64, :].rearrange(
                        "j (rr uu two) -> j two uu rr", two=2, uu=8)
                    rb_v = rb.rearrange(
                        "j (rr uu two) -> j two uu rr", two=2, uu=8)
                    o2_v = o2s[g][i].rearrange(
                        "q (u rr) -> q u rr", u=8)[:, :, 32 * a:32 * a + 32]
                    for par in range(2):
                        nc.vector.tensor_mul(
                            o2_v[64 * par:64 * par + 64],
                            pv_v[:, par], rb_v[:, par])
                    if i == 1:
                        pvs_by.pop((g, a))

                pending = None
                inter = []
                sched = []
                tail_parts = None
                rot = [0]

                def next_tag():
                    rot[0] += 1
                    return "stA" if rot[0] % 2 == 0 else "stB"

                units = [(g, a, gg, i) for g in range(NG) for a in range(4)
                         for gg in range(2 * a + 2) for i in range(2)]
                for ui, (g, a, gg, i) in enumerate(units):
                    if inter and sched and ui >= sched[0]:
                        sched.pop(0)
                        inter.pop(0)(next_tag())
                    if gg == 0 and i == 0:
                        pvs_by[(g, a)] = [
                            mmps.tile([65, 512], F32, tag="mm",
                                      name=f"pv{g}_{a}_{ii}")
                            for ii in range(2)]
                    diag = gg >= 2 * a
                    d = gg - 2 * a
                    sts = stps.tile([128, 1024], F32, tag=next_tag(),
                                    name=f"st{g}_{a}_{gg}_{i}")
                    for q2 in range(2):
                        bb = 2 * gg + q2
                        # diagonal blocks m=1,2 only need cols >= 128m (m=3
                        # would drop the free dim under 256 for no gain)
                        m = 2 * d + q2 if diag else 0
                        off = 128 * m if m in (1, 2) else 0
                        nc.tensor.matmul(
                            sts[:, q2 * 512 + off:(q2 + 1) * 512],
                            lhsT=kht2[g][64 * i:64 * i + 64,
                                         bb * 128:(bb + 1) * 128],
                            rhs=qht2[g][64 * i:64 * i + 64,
                                        a * 512 + off:(a + 1) * 512],
                            start=True, stop=True,
                        )
                    pt = ptp.tile([128, 1024], BF16, tag="pt",
                                  name=f"pt_{g}_{a}_{gg}_{i}")
                    if not diag:
                        nc.scalar.activation(pt, sts, EXP)
                    elif d == 0:
                        # one full-width exp (cols [512:640) are garbage the
                        # restricted PV never reads); strips masked after
                        nc.scalar.activation(pt, sts, EXP)
                        nc.vector.tensor_mul(
                            pt[:, 0:128], pt[:, 0:128], tri01)
                        nc.vector.tensor_mul(
                            pt[:, 640:768], pt[:, 640:768], tri01)
                    else:
                        # exp only the valid columns; zero the boundary
                        # strip's upper triangle with the 0/1 mask
                        for q2 in range(2):
                            m = 2 * d + q2
                            lo = q2 * 512 + 128 * m
                            hi = (q2 + 1) * 512
                            nc.scalar.activation(
                                pt[:, lo:hi], sts[:, lo:hi], EXP)
                            nc.vector.tensor_mul(
                                pt[:, lo:lo + 128],
                                pt[:, lo:lo + 128], tri01)
                    if pending is not None:
                        emit_pv(*pending)
                        pg, pa, pgg, pi, _ = pending
                        if pgg == 2 * pa + 1:
                            # this pair's PV chain just completed
                            emit_norm_one(pg, pa, pi)
                            if pg == 0 and pa == 3 and pi == 1:
                                inter = emit_y_parts(0, o2s[0], wo_sb, ypool)
                                sched = [ui + 6, ui + 10]
                            if pg == 1 and pa == 3 and pi == 0:
                                # overlap g1's first output-projection half
                                # with the final unit's PV + normalization
                                tail_parts = emit_y_parts(1, o2s[1], wo_sb,
                                                          ypool)
                                tail_parts[0]()
                    pending = (g, a, gg, i, pt)
                emit_pv(*pending)
                emit_norm_one(pending[0], pending[1], pending[3])
                tail_parts[1]()
                return o2s

            def emit_y_parts(g, o2, wo_sb, ypool):
                """Returns closures [part_jh0, part_jh1]; each emits half of
                the output projection so it can interleave with the next
                group's attention stream."""
                ysbs = [ypool.tile([128, 1024], F32, tag="ysb",
                                   name=f"ysb{g}_{i}") for i in range(2)]

                def part(i, tag=None):
                    for jh in range(2):
                        yps = mmps.tile([128, 512], F32, tag="mm",
                                        name=f"ypsum_{i}_{jh}")
                        for u in range(8):
                            nc.tensor.matmul(
                                yps,
                                lhsT=o2[i][:, u * 128:(u + 1) * 128],
                                rhs=wo_sb[:, u * 1024 + jh * 512:
                                          u * 1024 + (jh + 1) * 512],
                                start=(u == 0), stop=(u == 7),
                            )
                        nc.vector.tensor_add(
                            ysbs[i][:, jh * 512:(jh + 1) * 512], yps,
                            bias_sb[:, jh * 512:(jh + 1) * 512])
                        nc.sync.dma_start(
                            out=out[2 * g + i][:, jh * 512:(jh + 1) * 512],
                            in_=ysbs[i][:, jh * 512:(jh + 1) * 512])

                return [lambda tag=None: part(0), lambda tag=None: part(1)]

            with (
                tc.tile_pool(name="xtp", bufs=1) as xtp,
                tc.tile_pool(name="wp", bufs=1) as wp,
                tc.tile_pool(name="pwork", bufs=1) as pwork,
                tc.tile_pool(name="p2", bufs=1) as p2,
                tc.tile_pool(name="ptp", bufs=6) as ptp,
                tc.tile_pool(name="rp", bufs=2) as rp,
                tc.tile_pool(name="o2p", bufs=2) as o2p,
                tc.tile_pool(name="yp", bufs=2) as ypool,
            ):
                xb_sb = xtp.tile([128, 8, PPC, 128], BF16, tag="xb",
                                 name="xbsb")
                xv = xtb.rearrange("p (kc pr r) -> p kc pr r", kc=8, pr=PPC)
                for kc in range(8):
                    nc.scalar.dma_start(out=xb_sb[:, kc], in_=xv[:, kc])

                w_sbs = []
                for wi, (wparam, dt_) in enumerate(
                        ((wv, BF16), (wq, BF16), (wk, BF16))):
                    w_sb = wp.tile([128, 8, 1024], dt_, tag=f"w{wi}",
                                   name=f"w{wi}")
                    for kc in range(8):
                        nc.sync.dma_start(
                            out=w_sb[:, kc, :],
                            in_=wparam.rearrange(
                                "(c p) j -> p c j", p=128)[:, kc, :])
                    w_sbs.append(w_sb)
                wo_sb = p2.tile([128, 8 * 1024], BF16, tag="wo")
                nc.sync.dma_start(out=wo_sb, in_=wo[:])

                # group 0 runs as early as possible; group 1's projections
                # and pairs 2/3's V path are injected into attention(g0)'s
                # ACT-bound stream at psum-quiet unit indices
                pre_rot = [0]

                def pre_tag():
                    pre_rot[0] += 1
                    return "stA" if pre_rot[0] % 2 == 0 else "stB"

                q0 = qk_chunks(w_sbs[1], xb_sb, qht2, 0, "q")
                q1 = qk_chunks(w_sbs[1], xb_sb, qht2, 1, "q")
                k0 = qk_chunks(w_sbs[2], xb_sb, kht2, 0, "k")
                k1 = qk_chunks(w_sbs[2], xb_sb, kht2, 1, "k")
                for ch in v_chunks(w_sbs[0], xb_sb, pwork, [0, 1]):
                    ch()
                for ch in (q0[0], q1[0], q0[1], q1[1]):
                    ch(pre_tag())
                for ch in (k0[0], k1[0], k0[1], k1[1]):
                    ch(pre_tag())
                for ch in v_chunks(w_sbs[0], xb_sb, pwork, [2, 3]):
                    ch()
                nc.scalar.dma_start(out=bias_sb, in_=bias[:])

                emit_attention_fused(ptp, rp, o2p, ypool, wo_sb)

    nc.finalize()
    return nc


def _host_prep(input_seq_embs, W_Q, W_K, W_V, W_O, b_O):
    X = np.asarray(input_seq_embs, dtype=np.float32)
    WQ = np.asarray(W_Q, dtype=np.float32)
    WK = np.asarray(W_K, dtype=np.float32)
    WV = np.asarray(W_V, dtype=np.float32)
    WO = np.asarray(W_O, dtype=np.float32)
    bO = np.asarray(b_O, dtype=np.float32)

    import ml_dtypes
    bf16 = ml_dtypes.bfloat16

    wq_arr = np.ascontiguousarray(WQ.T).astype(bf16)
    wk_arr = np.ascontiguousarray(WK.T).astype(bf16)
    wv_arr = np.ascontiguousarray(WV.T).astype(bf16)
    # wo[64*parity + j2, 1024*u + jo] = W_O.T[64*(2u+parity) + j2, jo]
    wo_arr = np.ascontiguousarray(
        WO.T.reshape(8, 2, 64, 1024).transpose(1, 2, 0, 3).reshape(
            128, 8192)).astype(bf16)
    bias_arr = np.ascontiguousarray(
        np.broadcast_to(bO, (128, 1024)).astype(np.float32))

    in_maps = []
    for c in range(NCORES):
        # xt[p_, kc, pair, r] = X_pair[r, 128*kc + p_]
        xts = np.empty((128, 8, PPC, 128), dtype=np.float32)
        for p in range(PPC):
            g = PPC * c + p
            bb, hh = g // H, g % H
            Xs = X[bb, 128 * hh:128 * (hh + 1), :]      # (128 r, 1024 cin)
            xts[:, :, p, :] = Xs.T.reshape(8, 128, 128).transpose(1, 0, 2)
        xt_arr = np.ascontiguousarray(xts.reshape(128, 8 * PPC * 128))
        in_maps.append({
            "xtb": xt_arr.astype(bf16),
            "wq": wq_arr, "wk": wk_arr, "wv": wv_arr, "wo": wo_arr,
            "bias": bias_arr,
        })
    return in_maps


_CACHED_NC = None


def get_nc():
    global _CACHED_NC
    if _CACHED_NC is None:
        _CACHED_NC = build_nc()
    return _CACHED_NC


def kernel(**inputs) -> np.ndarray:
    nc = get_nc()
    in_maps = _host_prep(**inputs)
    res = run_bass_kernel_spmd(nc, in_maps, list(range(NCORES)))
    out = np.empty((B, L, D), dtype=np.float32)
    for c in range(NCORES):
        y = res.results[c]["out"]  # (4, 128, 1024)
        for p in range(PPC):
            g = PPC * c + p
            bb, hh = g // H, g % H
            out[bb, 128 * hh:128 * (hh + 1), :] = y[p]
    return out


# revision 8
# speedup vs baseline: 8.1863x; 1.0023x over previous
"""Trainium2 Bass kernel for nn_MultiHeadAttention_66322884984909.

Math (faithful to reference):
  Q = X @ W_Q.T reshaped (B, H, L, hd) via DIRECT reshape -> head h owns rows
  128h:128(h+1) of the projected (L, D) matrix, reinterpreted as (L=2048, hd=64).
  Heads are sequence-parallel: 32 (batch, head) pairs, 8 cores x 4 pairs.

Design:
  - Q/K computed PRE-TRANSPOSED: Qf^T = W_Q.T' @ X_s^T with output partitions
    = W-output-columns, batching a group's 2 pairs in the matmul free dim.
    psum quarters [(t-parity, j), (pair, r)] land in qht2/kht2[64i+j,
    s=16r+2c+parity] via strided DVE copies -- no DRAM shuffle bounce and no
    PE transposes for Q/K.  (psum start/stop act on whole 2KB banks: only the
    first/last quarter of a bank starts/stops its accumulation group.)
  - Causal masking without an inner-loop affine_select: on diagonal S tiles,
    exp and P@V read only columns q >= 128m of each key block; the 128-wide
    boundary strip is zeroed by one bf16 DVE multiply with a constant 0/1
    triangle.  The m=0 block is full width, so the psum accumulation group
    start stays uniform (bb == 0).
  - Attention is software-pipelined per unit (a-chunk, key-block-pair, pair):
    S(n+1) is emitted BEFORE PV(n) so the PE never idles on the exp latency;
    sts psum tiles double-buffer via a shared stA/stB tag rotation.
    Softmax row sums ride along as a 65th ones-column of V; normalization
    (reciprocal + partition_broadcast + strided muls) scatters O directly
    into the o2 layout [(s-parity, j2), (u, r)].
  - Output projection contracts 128-deep (8 accumulating matmuls per 512-col
    half), split into per-pair closures interleaved into group 1's
    attention stream.  Both groups' attention form ONE fused pipeline, so
    group 0's drain overlaps group 1's ramp-up.  W_V loads first and the
    descriptor-heavy V shuffle-scatter overlaps the W_Q/W_K streams;
    dummy matmuls/exp ramp the PE p-state and activation table during the
    initial DMA wait (kc is the outer loop so matmuls chase the W DMAs).
  - bf16 inputs and weights throughout (X, W_Q, W_K, W_V, W_O, the V DRAM
    bounce, exp output P, O): measured 1.05e-2 relative error against the
    2e-2 gate (softmax renormalization cancels most of the bf16 logit
    error); S and the psum accumulations stay fp32.  Halves the weight
    DMA lead-in that gates the attention start.
  - No max-subtraction in softmax: logits ~ N(0, 64); exp stays finite in
    fp32.  All fp32 matmuls run as fp32r with free dim >= 256 (full rate).

Cost-model (TimelineSim) total: 158.5 us vs 248.9 us for the v1 baseline.
"""

import numpy as np

import concourse.bass as bass
from concourse import bacc
import concourse.mybir as mybir
import concourse.tile as tile
from concourse.bass_utils import run_bass_kernel_spmd

F32 = mybir.dt.float32
F32R = mybir.dt.float32r
BF16 = mybir.dt.bfloat16
EXP = mybir.ActivationFunctionType.Exp

B, L, D = 2, 2048, 1024
H, HD = 16, 64
NCORES = 8
PPC = 4   # pairs per core
NG = 2    # groups of 2 pairs
NEG = -1.0e30


def build_nc(repeat=1):
    nc = bacc.Bacc(trn_type="TRN2", target_bir_lowering=False, debug=False)

    # xtb[p_, kc*512 + 128*pair + r] = X_pair[r, 128*kc + p_] (bf16)
    xtb = nc.declare_dram_parameter("xtb", [128, 8 * PPC * 128], BF16,
                                    isOutput=False)
    wq = nc.declare_dram_parameter("wq", [1024, 1024], BF16, isOutput=False)
    wk = nc.declare_dram_parameter("wk", [1024, 1024], BF16, isOutput=False)
    wv = nc.declare_dram_parameter("wv", [1024, 1024], BF16, isOutput=False)
    # wo[64*parity + j2, 1024*u + jo] = W_O.T[64*(2u+parity) + j2, jo]
    wo = nc.declare_dram_parameter("wo", [128, 8 * 1024], BF16, isOutput=False)
    bias = nc.declare_dram_parameter("bias", [128, 1024], F32, isOutput=False)
    out = nc.declare_dram_parameter("out", [PPC, 128, 1024], BF16,
                                    isOutput=True)
    vsh = nc.dram_tensor("vsh", [PPC, 128, 1040], BF16)

    with tile.TileContext(nc) as tc:
      for _rep in range(repeat):
        with (
            tc.tile_pool(name="consts", bufs=1) as consts,
            tc.tile_pool(name="headt", bufs=1) as headt,
            tc.tile_pool(name="mmps", bufs=4, space="PSUM") as mmps,
            tc.tile_pool(name="stps", bufs=1, space="PSUM") as stps,
        ):
            bias_sb = consts.tile([128, 1024], F32)
            # tri01[k, q'] = 1.0 if q' >= k else 0.0 (strip causal mask)
            tri01 = consts.tile([128, 128], BF16)
            nc.gpsimd.memset(tri01, 1.0)
            # dummy exp to preload the activation table during the
            # projection phase (the lazy load costs 1.3us otherwise)
            warm = consts.tile([128, 1], F32, tag="warm")
            nc.gpsimd.memset(warm, 0.0)
            nc.scalar.activation(warm, warm, EXP)
            # dummy matmuls ramp the PE p-state (full clock needs ~3us of
            # continuous execution) while the first weight DMAs stream in
            wmm = consts.tile([128, 256], F32, tag="wmm")
            nc.gpsimd.memset(wmm, 0.0)
            for _w in range(6):
                pw = mmps.tile([128, 256], F32, tag="mm", name=f"warmmm{_w}")
                nc.tensor.matmul(pw, lhsT=wmm[:, 0:128], rhs=wmm,
                                 start=True, stop=True)
            nc.gpsimd.affine_select(
                out=tri01, in_=tri01,
                compare_op=mybir.AluOpType.is_ge,
                fill=0.0,
                base=0,
                pattern=[[1, 128]],
                channel_multiplier=-1,
            )

            # qht2/kht2[g]: [64*i + j, s] for pair 2g+i  (j = head dim)
            qht2 = [headt.tile([128, 2048], F32R, tag=f"qht{g}", name=f"qht{g}")
                    for g in range(NG)]
            kht2 = [headt.tile([128, 2048], F32R, tag=f"kht{g}", name=f"kht{g}")
                    for g in range(NG)]
            # vh[p]: [s-in-block, 65*bb + j] with ones column at j=64
            vh = [headt.tile([128, 16 * 65], BF16, tag=f"vh{p}", name=f"vh{p}")
                  for p in range(PPC)]

            def qk_chunks(w_sb, xq_sb, dsts, g, wname):
                """Transposed projection, as 4 chunk-closures per (W, g).

                Chunk (half, bank): one psum tile [128, 512] = 2 c-quarters
                (c = 4*half + 2*bank + cq).  Quarter c holds [(parity, j),
                (i, r)] with W-output column 128c + 64*parity + j; copies land
                in dsts[g][64i + j, s = 16r + 2c + parity].  psum start/stop
                act on whole 2KB banks, so only the first quarter starts and
                only the last stops the accumulation group.
                """
                def mk(half):
                    def emit(tag="stA"):
                        pg = stps.tile(
                            [128, 1024], F32, tag=tag,
                            name=f"prj_{wname}_{g}_{half}")
                        for kc in range(8):
                            for c4 in range(4):
                                c = 4 * half + c4
                                # psum start/stop act on whole 2KB banks: a
                                # bank spans two 256-col quarters, so only the
                                # first quarter starts / last quarter stops
                                nc.tensor.matmul(
                                    pg[:, 256 * c4:256 * c4 + 256],
                                    lhsT=w_sb[:, kc, c * 128:(c + 1) * 128],
                                    rhs=xq_sb[:, kc, 2 * g:2 * g + 2, :],
                                    start=(kc == 0 and c4 % 2 == 0),
                                    stop=(kc == 7 and c4 % 2 == 1),
                                )
                        v = dsts[g].rearrange(
                            "(i j) (r c two) -> i j two c r", i=2, c=8, two=2)
                        pv4 = pg.rearrange(
                            "p (c4 two r) -> p c4 two r", c4=4, two=2)
                        for parity in range(2):
                            for i in range(2):
                                nc.vector.tensor_copy(
                                    v[i, :, parity, 4 * half:4 * half + 4, :],
                                    pv4[64 * parity:64 * parity + 64, :, i, :])
                    return emit
                return [mk(h) for h in range(2)]

            def v_chunks(wv_sb, xb_sb, pwork, pairs):
                """V projection + DRAM shuffle bounce, one closure per
                (pair, jh-half)."""
                nats = {}

                def mk(p, jh):
                    def emit(tag=None):
                        if jh == 0:
                            nats[p] = pwork.tile([128, 1024], BF16, tag="natv",
                                                 bufs=2, name=f"natv{p}")
                        nat = nats[p]
                        ps = mmps.tile([128, 512], F32, tag="mm",
                                       name="projps")
                        for kc in range(8):
                            nc.tensor.matmul(
                                ps,
                                lhsT=xb_sb[:, kc, p, :],
                                rhs=wv_sb[:, kc, jh * 512:(jh + 1) * 512],
                                start=(kc == 0), stop=(kc == 7),
                            )
                        nc.vector.tensor_copy(
                            nat[:, jh * 512:(jh + 1) * 512], ps)
                        if jh == 1:
                            shr = vsh[p].rearrange(
                                "(il pp2) (t j) -> t il pp2 j",
                                il=8, t=16)[:, :, :, 0:64]
                            nc.gpsimd.dma_start(out=shr, in_=nat[:])
                            nc.scalar.dma_start(out=vh[p][:], in_=vsh[p])
                            nc.gpsimd.memset(
                                vh[p].rearrange(
                                    "q (b c) -> q b c", c=65)[:, :, 64],
                                1.0)  # ones column at 65b+64
                    return emit
                return [mk(p, jh) for p in pairs for jh in range(2)]

            def emit_attention_fused(ptp, rp, o2p, ypool, wo_sb):
                """Both groups' attention as ONE software-pipelined stream:
                the PV matmuls for unit n are emitted AFTER the S matmuls of
                unit n+1 (psum stA/stB rotate at depth 2), and the pipeline
                crosses the group boundary so group 0's drain overlaps group
                1's ramp-up.  Group 0's output projection is injected a few
                units after its last normalization."""
                o2s = {g: [o2p.tile([128, 8 * 128], BF16, tag=f"o2_{i}",
                                    name=f"o2_{g}_{i}") for i in range(2)]
                       for g in range(NG)}
                pvs_by = {}

                def emit_pv(g, a, gg, i, pt):
                    diag = gg >= 2 * a
                    d = gg - 2 * a
                    for q2 in range(2):
                        bb = 2 * gg + q2
                        if not diag:
                            nc.tensor.matmul(
                                pvs_by[(g, a)][i],
                                lhsT=vh[2 * g + i][:, bb * 65:bb * 65 + 65],
                                rhs=pt[:, q2 * 512:(q2 + 1) * 512],
                                start=(bb == 0),
                                stop=(bb == 4 * a + 3),
                            )
                        else:
                            m = 2 * d + q2
                            lo = q2 * 512 + 128 * m
                            nc.tensor.matmul(
                                pvs_by[(g, a)][i][:, 128 * m:512],
                                lhsT=vh[2 * g + i][:, bb * 65:bb * 65 + 65],
                                rhs=pt[:, lo:(q2 + 1) * 512],
                                start=(bb == 0),
                                stop=(bb == 4 * a + 3),
                                skip_group_check=True,
                            )

                def emit_norm_one(g, a, i):
                    pvs = pvs_by[(g, a)]
                    r1 = rp.tile([1, 512], F32, tag="r1", name="r1_t")
                    nc.vector.reciprocal(r1, pvs[i][64:65, :])
                    rb = rp.tile([64, 512], F32, tag="rb", name="rb_t")
                    nc.gpsimd.partition_broadcast(rb, r1)
                    # o2[64*par + j2, 128u + 32a + r'] =
                    #     pvs[j2, 16r' + 2u + par] * rb[...]
                    pv_v = pvs[i][0:64, :].rearrange(
                        "j (rr uu two) -> j two uu rr", two=2, uu=8)
                    rb_v = rb.rearrange(
                        "j (rr uu two) -> j two uu rr", two=2, uu=8)
                    o2_v = o2s[g][i].rearrange(
                        "q (u rr) -> q u rr", u=8)[:, :, 32 * a:32 * a + 32]
                    for par in range(2):
                        nc.vector.tensor_mul(
                            o2_v[64 * par:64 * par + 64],
                            pv_v[:, par], rb_v[:, par])
                    if i == 1:
                        pvs_by.pop((g, a))

                pending = None
                inter = []
                sched = []
                tail_parts = None
                rot = [0]

                def next_tag():
                    rot[0] += 1
                    return "stA" if rot[0] % 2 == 0 else "stB"

                units = [(g, a, gg, i) for g in range(NG) for a in range(4)
                         for gg in range(2 * a + 2) for i in range(2)]
                for ui, (g, a, gg, i) in enumerate(units):
                    if inter and sched and ui >= sched[0]:
                        sched.pop(0)
                        inter.pop(0)(next_tag())
                    if gg == 0 and i == 0:
                        pvs_by[(g, a)] = [
                            mmps.tile([65, 512], F32, tag="mm",
                                      name=f"pv{g}_{a}_{ii}")
                            for ii in range(2)]
                    diag = gg >= 2 * a
                    d = gg - 2 * a
                    sts = stps.tile([128, 1024], F32, tag=next_tag(),
                                    name=f"st{g}_{a}_{gg}_{i}")
                    for q2 in range(2):
                        bb = 2 * gg + q2
                        # diagonal blocks m=1,2 only need cols >= 128m (m=3
                        # would drop the free dim under 256 for no gain)
                        m = 2 * d + q2 if diag else 0
                        off = 128 * m if m in (1, 2) else 0
                        nc.tensor.matmul(
                            sts[:, q2 * 512 + off:(q2 + 1) * 512],
                            lhsT=kht2[g][64 * i:64 * i + 64,
                                         bb * 128:(bb + 1) * 128],
                            rhs=qht2[g][64 * i:64 * i + 64,
                                        a * 512 + off:(a + 1) * 512],
                            start=True, stop=True,
                        )
                    pt = ptp.tile([128, 1024], BF16, tag="pt",
                                  name=f"pt_{g}_{a}_{gg}_{i}")
                    if not diag:
                        nc.scalar.activation(pt, sts, EXP)
                    elif d == 0:
                        # one full-width exp (cols [512:640) are garbage the
                        # restricted PV never reads); strips masked after
                        nc.scalar.activation(pt, sts, EXP)
                        nc.vector.tensor_mul(
                            pt[:, 0:128], pt[:, 0:128], tri01)
                        nc.vector.tensor_mul(
                            pt[:, 640:768], pt[:, 640:768], tri01)
                    else:
                        # exp only the valid columns; zero the boundary
                        # strip's upper triangle with the 0/1 mask
                        for q2 in range(2):
                            m = 2 * d + q2
                            lo = q2 * 512 + 128 * m
                            hi = (q2 + 1) * 512
                            nc.scalar.activation(
                                pt[:, lo:hi], sts[:, lo:hi], EXP)
                            nc.vector.tensor_mul(
                                pt[:, lo:lo + 128],
                                pt[:, lo:lo + 128], tri01)
                    if pending is not None:
                        emit_pv(*pending)
                        pg, pa, pgg, pi, _ = pending
                        if pgg == 2 * pa + 1:
                            # this pair's PV chain just completed
                            emit_norm_one(pg, pa, pi)
                            if pg == 0 and pa == 3 and pi == 1:
                                inter = emit_y_parts(0, o2s[0], wo_sb, ypool)
                                sched = [ui + 6, ui + 10]
                            if pg == 1 and pa == 3 and pi == 0:
                                # overlap g1's first output-projection half
                                # with the final unit's PV + normalization
                                tail_parts = emit_y_parts(1, o2s[1], wo_sb,
                                                          ypool)
                                tail_parts[0]()
                    pending = (g, a, gg, i, pt)
                emit_pv(*pending)
                emit_norm_one(pending[0], pending[1], pending[3])
                tail_parts[1]()
                return o2s

            def emit_y_parts(g, o2, wo_sb, ypool):
                """Returns closures [part_jh0, part_jh1]; each emits half of
                the output projection so it can interleave with the next
                group's attention stream."""
                ysbs = [ypool.tile([128, 1024], BF16, tag="ysb",
                                   name=f"ysb{g}_{i}") for i in range(2)]

                def part(i, tag=None):
                    for jh in range(2):
                        yps = mmps.tile([128, 512], F32, tag="mm",
                                        name=f"ypsum_{i}_{jh}")
                        for u in range(8):
                            nc.tensor.matmul(
                                yps,
                                lhsT=o2[i][:, u * 128:(u + 1) * 128],
                                rhs=wo_sb[:, u * 1024 + jh * 512:
                                          u * 1024 + (jh + 1) * 512],
                                start=(u == 0), stop=(u == 7),
                            )
                        nc.vector.tensor_add(
                            ysbs[i][:, jh * 512:(jh + 1) * 512], yps,
                            bias_sb[:, jh * 512:(jh + 1) * 512])
                        nc.sync.dma_start(
                            out=out[2 * g + i][:, jh * 512:(jh + 1) * 512],
                            in_=ysbs[i][:, jh * 512:(jh + 1) * 512])

                return [lambda tag=None: part(0), lambda tag=None: part(1)]

            with (
                tc.tile_pool(name="xtp", bufs=1) as xtp,
                tc.tile_pool(name="wp", bufs=1) as wp,
                tc.tile_pool(name="pwork", bufs=1) as pwork,
                tc.tile_pool(name="p2", bufs=1) as p2,
                tc.tile_pool(name="ptp", bufs=6) as ptp,
                tc.tile_pool(name="rp", bufs=2) as rp,
                tc.tile_pool(name="o2p", bufs=2) as o2p,
                tc.tile_pool(name="yp", bufs=2) as ypool,
            ):
                xb_sb = xtp.tile([128, 8, PPC, 128], BF16, tag="xb",
                                 name="xbsb")
                xv = xtb.rearrange("p (kc pr r) -> p kc pr r", kc=8, pr=PPC)
                for kc in range(8):
                    nc.scalar.dma_start(out=xb_sb[:, kc], in_=xv[:, kc])

                w_sbs = []
                for wi, (wparam, dt_) in enumerate(
                        ((wv, BF16), (wq, BF16), (wk, BF16))):
                    w_sb = wp.tile([128, 8, 1024], dt_, tag=f"w{wi}",
                                   name=f"w{wi}")
                    for kc in range(8):
                        nc.sync.dma_start(
                            out=w_sb[:, kc, :],
                            in_=wparam.rearrange(
                                "(c p) j -> p c j", p=128)[:, kc, :])
                    w_sbs.append(w_sb)
                wo_sb = p2.tile([128, 8 * 1024], BF16, tag="wo")
                nc.sync.dma_start(out=wo_sb, in_=wo[:])

                # group 0 runs as early as possible; group 1's projections
                # and pairs 2/3's V path are injected into attention(g0)'s
                # ACT-bound stream at psum-quiet unit indices
                pre_rot = [0]

                def pre_tag():
                    pre_rot[0] += 1
                    return "stA" if pre_rot[0] % 2 == 0 else "stB"

                q0 = qk_chunks(w_sbs[1], xb_sb, qht2, 0, "q")
                q1 = qk_chunks(w_sbs[1], xb_sb, qht2, 1, "q")
                k0 = qk_chunks(w_sbs[2], xb_sb, kht2, 0, "k")
                k1 = qk_chunks(w_sbs[2], xb_sb, kht2, 1, "k")
                for ch in v_chunks(w_sbs[0], xb_sb, pwork, [0, 1]):
                    ch()
                for ch in (q0[0], q1[0], q0[1], q1[1]):
                    ch(pre_tag())
                for ch in (k0[0], k1[0], k0[1], k1[1]):
                    ch(pre_tag())
                for ch in v_chunks(w_sbs[0], xb_sb, pwork, [2, 3]):
                    ch()
                nc.scalar.dma_start(out=bias_sb, in_=bias[:])

                emit_attention_fused(ptp, rp, o2p, ypool, wo_sb)

    nc.finalize()
    return nc


def _host_prep(input_seq_embs, W_Q, W_K, W_V, W_O, b_O):
    X = np.asarray(input_seq_embs, dtype=np.float32)
    WQ = np.asarray(W_Q, dtype=np.float32)
    WK = np.asarray(W_K, dtype=np.float32)
    WV = np.asarray(W_V, dtype=np.float32)
    WO = np.asarray(W_O, dtype=np.float32)
    bO = np.asarray(b_O, dtype=np.float32)

    import ml_dtypes
    bf16 = ml_dtypes.bfloat16

    wq_arr = np.ascontiguousarray(WQ.T).astype(bf16)
    wk_arr = np.ascontiguousarray(WK.T).astype(bf16)
    wv_arr = np.ascontiguousarray(WV.T).astype(bf16)
    # wo[64*parity + j2, 1024*u + jo] = W_O.T[64*(2u+parity) + j2, jo]
    wo_arr = np.ascontiguousarray(
        WO.T.reshape(8, 2, 64, 1024).transpose(1, 2, 0, 3).reshape(
            128, 8192)).astype(bf16)
    bias_arr = np.ascontiguousarray(
        np.broadcast_to(bO, (128, 1024)).astype(np.float32))

    in_maps = []
    for c in range(NCORES):
        # xt[p_, kc, pair, r] = X_pair[r, 128*kc + p_]
        xts = np.empty((128, 8, PPC, 128), dtype=np.float32)
        for p in range(PPC):
            g = PPC * c + p
            bb, hh = g // H, g % H
            Xs = X[bb, 128 * hh:128 * (hh + 1), :]      # (128 r, 1024 cin)
            xts[:, :, p, :] = Xs.T.reshape(8, 128, 128).transpose(1, 0, 2)
        xt_arr = np.ascontiguousarray(xts.reshape(128, 8 * PPC * 128))
        in_maps.append({
            "xtb": xt_arr.astype(bf16),
            "wq": wq_arr, "wk": wk_arr, "wv": wv_arr, "wo": wo_arr,
            "bias": bias_arr,
        })
    return in_maps


_CACHED_NC = None


def get_nc():
    global _CACHED_NC
    if _CACHED_NC is None:
        _CACHED_NC = build_nc()
    return _CACHED_NC


def kernel(**inputs) -> np.ndarray:
    nc = get_nc()
    in_maps = _host_prep(**inputs)
    res = run_bass_kernel_spmd(nc, in_maps, list(range(NCORES)))
    out = np.empty((B, L, D), dtype=np.float32)
    for c in range(NCORES):
        y = np.asarray(res.results[c]["out"],
                       dtype=np.float32)  # (4, 128, 1024)
        for p in range(PPC):
            g = PPC * c + p
            bb, hh = g // H, g % H
            out[bb, 128 * hh:128 * (hh + 1), :] = y[p]
    return out


# revision 9
# speedup vs baseline: 8.3814x; 1.0238x over previous
"""Trainium2 Bass kernel for nn_MultiHeadAttention_66322884984909.

Math (faithful to reference):
  Q = X @ W_Q.T reshaped (B, H, L, hd) via DIRECT reshape -> head h owns rows
  128h:128(h+1) of the projected (L, D) matrix, reinterpreted as (L=2048, hd=64).
  Heads are sequence-parallel: 32 (batch, head) pairs, 8 cores x 4 pairs.

Design:
  - Q/K computed PRE-TRANSPOSED: Qf^T = W_Q.T' @ X_s^T with output partitions
    = W-output-columns, batching a group's 2 pairs in the matmul free dim.
    psum quarters [(t-parity, j), (pair, r)] land in qht2/kht2[64i+j,
    s=16r+2c+parity] via strided DVE copies -- no DRAM shuffle bounce and no
    PE transposes for Q/K.  (psum start/stop act on whole 2KB banks: only the
    first/last quarter of a bank starts/stops its accumulation group.)
  - Causal masking without an inner-loop affine_select: on diagonal S tiles,
    exp and P@V read only columns q >= 128m of each key block; the 128-wide
    boundary strip is zeroed by one bf16 DVE multiply with a constant 0/1
    triangle.  The m=0 block is full width, so the psum accumulation group
    start stays uniform (bb == 0).
  - Attention is software-pipelined per unit (a-chunk, key-block-pair, pair):
    S(n+1) is emitted BEFORE PV(n) so the PE never idles on the exp latency;
    sts psum tiles double-buffer via a shared stA/stB tag rotation.
    Softmax row sums ride along as a 65th ones-column of V; normalization
    (reciprocal + partition_broadcast + strided muls) scatters O directly
    into the o2 layout [(s-parity, j2), (u, r)].
  - Output projection contracts 128-deep (8 accumulating matmuls per 512-col
    half), split into per-pair closures interleaved into group 1's
    attention stream.  Both groups' attention form ONE fused pipeline, so
    group 0's drain overlaps group 1's ramp-up.  W_V loads first and the
    descriptor-heavy V shuffle-scatter overlaps the W_Q/W_K streams;
    dummy matmuls/exp ramp the PE p-state and activation table during the
    initial DMA wait (kc is the outer loop so matmuls chase the W DMAs).
  - bf16 inputs and weights throughout (X, W_Q, W_K, W_V, W_O, the V DRAM
    bounce, exp output P, O): measured 1.05e-2 relative error against the
    2e-2 gate (softmax renormalization cancels most of the bf16 logit
    error); S and the psum accumulations stay fp32.  Halves the weight
    DMA lead-in that gates the attention start.
  - No max-subtraction in softmax: logits ~ N(0, 64); exp stays finite in
    fp32.  All fp32 matmuls run as fp32r with free dim >= 256 (full rate).

Cost-model (TimelineSim) total: 158.8 us vs 248.9 us for the v1 baseline.
"""

import numpy as np

import concourse.bass as bass
from concourse import bacc
import concourse.mybir as mybir
import concourse.tile as tile
from concourse.bass_utils import run_bass_kernel_spmd

F32 = mybir.dt.float32
F32R = mybir.dt.float32r
BF16 = mybir.dt.bfloat16
EXP = mybir.ActivationFunctionType.Exp

B, L, D = 2, 2048, 1024
H, HD = 16, 64
NCORES = 8
PPC = 4   # pairs per core
NG = 2    # groups of 2 pairs
NEG = -1.0e30


def build_nc(repeat=1):
    nc = bacc.Bacc(trn_type="TRN2", target_bir_lowering=False, debug=False)

    # xtb[p_, kc*512 + 128*pair + r] = X_pair[r, 128*kc + p_] (bf16)
    xtb = nc.declare_dram_parameter("xtb", [128, 8 * PPC * 128], BF16,
                                    isOutput=False)
    wq = nc.declare_dram_parameter("wq", [1024, 1024], BF16, isOutput=False)
    wk = nc.declare_dram_parameter("wk", [1024, 1024], BF16, isOutput=False)
    wv = nc.declare_dram_parameter("wv", [1024, 1024], BF16, isOutput=False)
    # wo[64*parity + j2, 1024*u + jo] = W_O.T[64*(2u+parity) + j2, jo]
    wo = nc.declare_dram_parameter("wo", [128, 8 * 1024], BF16, isOutput=False)
    bias = nc.declare_dram_parameter("bias", [128, 1024], F32, isOutput=False)
    out = nc.declare_dram_parameter("out", [PPC, 128, 1024], BF16,
                                    isOutput=True)
    vsh = nc.dram_tensor("vsh", [PPC, 128, 1040], BF16)

    with tile.TileContext(nc) as tc:
      for _rep in range(repeat):
        with (
            tc.tile_pool(name="consts", bufs=1) as consts,
            tc.tile_pool(name="headt", bufs=1) as headt,
            tc.tile_pool(name="mmps", bufs=4, space="PSUM") as mmps,
            tc.tile_pool(name="stps", bufs=1, space="PSUM") as stps,
        ):
            bias_sb = consts.tile([128, 1024], F32)
            # tri01[k, q'] = 1.0 if q' >= k else 0.0 (strip causal mask)
            tri01 = consts.tile([128, 128], BF16)
            nc.gpsimd.memset(tri01, 1.0)
            # dummy exp to preload the activation table during the
            # projection phase (the lazy load costs 1.3us otherwise)
            warm = consts.tile([128, 1], F32, tag="warm")
            nc.gpsimd.memset(warm, 0.0)
            nc.scalar.activation(warm, warm, EXP)
            # dummy matmuls ramp the PE p-state (full clock needs ~3us of
            # continuous execution) while the first weight DMAs stream in
            wmm = consts.tile([128, 256], F32, tag="wmm")
            nc.gpsimd.memset(wmm, 0.0)
            for _w in range(6):
                pw = mmps.tile([128, 256], F32, tag="mm", name=f"warmmm{_w}")
                nc.tensor.matmul(pw, lhsT=wmm[:, 0:128], rhs=wmm,
                                 start=True, stop=True)
            nc.gpsimd.affine_select(
                out=tri01, in_=tri01,
                compare_op=mybir.AluOpType.is_ge,
                fill=0.0,
                base=0,
                pattern=[[1, 128]],
                channel_multiplier=-1,
            )

            # qht2/kht2[g]: [64*i + j, s] for pair 2g+i  (j = head dim)
            qht2 = [headt.tile([128, 2048], F32R, tag=f"qht{g}", name=f"qht{g}")
                    for g in range(NG)]
            kht2 = [headt.tile([128, 2048], F32R, tag=f"kht{g}", name=f"kht{g}")
                    for g in range(NG)]
            # vh[p]: [s-in-block, 65*bb + j] with ones column at j=64
            vh = [headt.tile([128, 16 * 65], BF16, tag=f"vh{p}", name=f"vh{p}")
                  for p in range(PPC)]

            def qk_chunks(w_sb, xq_sb, dsts, g, wname):
                """Transposed projection, as 4 chunk-closures per (W, g).

                Chunk (half, bank): one psum tile [128, 512] = 2 c-quarters
                (c = 4*half + 2*bank + cq).  Quarter c holds [(parity, j),
                (i, r)] with W-output column 128c + 64*parity + j; copies land
                in dsts[g][64i + j, s = 16r + 2c + parity].  psum start/stop
                act on whole 2KB banks, so only the first quarter starts and
                only the last stops the accumulation group.
                """
                def mk(half):
                    def emit(tag="stA"):
                        pg = stps.tile(
                            [128, 1024], F32, tag=tag,
                            name=f"prj_{wname}_{g}_{half}")
                        for kc in range(8):
                            for c4 in range(4):
                                c = 4 * half + c4
                                # psum start/stop act on whole 2KB banks: a
                                # bank spans two 256-col quarters, so only the
                                # first quarter starts / last quarter stops
                                nc.tensor.matmul(
                                    pg[:, 256 * c4:256 * c4 + 256],
                                    lhsT=w_sb[:, kc, c * 128:(c + 1) * 128],
                                    rhs=xq_sb[:, kc, 2 * g:2 * g + 2, :],
                                    start=(kc == 0 and c4 % 2 == 0),
                                    stop=(kc == 7 and c4 % 2 == 1),
                                )
                        v = dsts[g].rearrange(
                            "(i j) (r c two) -> i j two c r", i=2, c=8, two=2)
                        pv4 = pg.rearrange(
                            "p (c4 two r) -> p c4 two r", c4=4, two=2)
                        for parity in range(2):
                            for i in range(2):
                                nc.vector.tensor_copy(
                                    v[i, :, parity, 4 * half:4 * half + 4, :],
                                    pv4[64 * parity:64 * parity + 64, :, i, :])
                    return emit
                return [mk(h) for h in range(2)]

            def v_chunks(wv_sb, xb_sb, pwork, pairs):
                """V projection + DRAM shuffle bounce, one closure per
                (pair, jh-half)."""
                nats = {}

                def mk(p, jh):
                    def emit(tag=None):
                        if jh == 0:
                            nats[p] = pwork.tile([128, 1024], BF16, tag="natv",
                                                 bufs=2, name=f"natv{p}")
                        nat = nats[p]
                        ps = mmps.tile([128, 512], F32, tag="mm",
                                       name="projps")
                        for kc in range(8):
                            nc.tensor.matmul(
                                ps,
                                lhsT=xb_sb[:, kc, p, :],
                                rhs=wv_sb[:, kc, jh * 512:(jh + 1) * 512],
                                start=(kc == 0), stop=(kc == 7),
                            )
                        nc.vector.tensor_copy(
                            nat[:, jh * 512:(jh + 1) * 512], ps)
                        if jh == 1:
                            shr = vsh[p].rearrange(
                                "(il pp2) (t j) -> t il pp2 j",
                                il=8, t=16)[:, :, :, 0:64]
                            nc.gpsimd.dma_start(out=shr, in_=nat[:])
                            nc.scalar.dma_start(out=vh[p][:], in_=vsh[p])
                            nc.gpsimd.memset(
                                vh[p].rearrange(
                                    "q (b c) -> q b c", c=65)[:, :, 64],
                                1.0)  # ones column at 65b+64
                    return emit
                return [mk(p, jh) for p in pairs for jh in range(2)]

            def emit_attention_fused(ptp, rp, o2p, ypool, wo_sb):
                """Both groups' attention as ONE software-pipelined stream:
                the PV matmuls for unit n are emitted AFTER the S matmuls of
                unit n+1 (psum stA/stB rotate at depth 2), and the pipeline
                crosses the group boundary so group 0's drain overlaps group
                1's ramp-up.  Group 0's output projection is injected a few
                units after its last normalization."""
                o2s = {g: [o2p.tile([128, 8 * 128], BF16, tag=f"o2_{i}",
                                    name=f"o2_{g}_{i}") for i in range(2)]
                       for g in range(NG)}
                pvs_by = {}

                def emit_pv(g, a, gg, i, pt):
                    diag = gg >= 2 * a
                    d = gg - 2 * a
                    for q2 in range(2):
                        bb = 2 * gg + q2
                        if not diag:
                            nc.tensor.matmul(
                                pvs_by[(g, a)][i],
                                lhsT=vh[2 * g + i][:, bb * 65:bb * 65 + 65],
                                rhs=pt[:, q2 * 512:(q2 + 1) * 512],
                                start=(bb == 0),
                                stop=(bb == 4 * a + 3),
                            )
                        else:
                            m = 2 * d + q2
                            lo = q2 * 512 + 128 * m
                            nc.tensor.matmul(
                                pvs_by[(g, a)][i][:, 128 * m:512],
                                lhsT=vh[2 * g + i][:, bb * 65:bb * 65 + 65],
                                rhs=pt[:, lo:(q2 + 1) * 512],
                                start=(bb == 0),
                                stop=(bb == 4 * a + 3),
                                skip_group_check=True,
                            )

                def emit_norm_one(g, a, i):
                    pvs = pvs_by[(g, a)]
                    r1 = rp.tile([1, 512], F32, tag="r1", name="r1_t")
                    nc.vector.reciprocal(r1, pvs[i][64:65, :])
                    rb = rp.tile([64, 512], F32, tag="rb", name="rb_t")
                    nc.gpsimd.partition_broadcast(rb, r1)
                    # o2[64*par + j2, 128u + 32a + r'] =
                    #     pvs[j2, 16r' + 2u + par] * rb[...]
                    pv_v = pvs[i][0:64, :].rearrange(
                        "j (rr uu two) -> j two uu rr", two=2, uu=8)
                    rb_v = rb.rearrange(
                        "j (rr uu two) -> j two uu rr", two=2, uu=8)
                    o2_v = o2s[g][i].rearrange(
                        "q (u rr) -> q u rr", u=8)[:, :, 32 * a:32 * a + 32]
                    for par in range(2):
                        nc.vector.tensor_mul(
                            o2_v[64 * par:64 * par + 64],
                            pv_v[:, par], rb_v[:, par])
                    if i == 1:
                        pvs_by.pop((g, a))

                pending = None
                inter = []
                sched = []
                tail_parts = None
                rot = [0]

                def next_tag():
                    rot[0] += 1
                    return "stA" if rot[0] % 2 == 0 else "stB"

                units = [(g, a, gg, i) for g in range(NG) for a in range(4)
                         for gg in range(2 * a + 2) for i in range(2)]
                for ui, (g, a, gg, i) in enumerate(units):
                    if inter and sched and ui >= sched[0]:
                        sched.pop(0)
                        inter.pop(0)(next_tag())
                    if gg == 0 and i == 0:
                        pvs_by[(g, a)] = [
                            mmps.tile([65, 512], F32, tag="mm",
                                      name=f"pv{g}_{a}_{ii}")
                            for ii in range(2)]
                    diag = gg >= 2 * a
                    d = gg - 2 * a
                    sts = stps.tile([128, 1024], F32, tag=next_tag(),
                                    name=f"st{g}_{a}_{gg}_{i}")
                    for q2 in range(2):
                        bb = 2 * gg + q2
                        # diagonal blocks m=1,2 only need cols >= 128m (m=3
                        # would drop the free dim under 256 for no gain)
                        m = 2 * d + q2 if diag else 0
                        off = 128 * m if m in (1, 2) else 0
                        nc.tensor.matmul(
                            sts[:, q2 * 512 + off:(q2 + 1) * 512],
                            lhsT=kht2[g][64 * i:64 * i + 64,
                                         bb * 128:(bb + 1) * 128],
                            rhs=qht2[g][64 * i:64 * i + 64,
                                        a * 512 + off:(a + 1) * 512],
                            start=True, stop=True,
                        )
                    pt = ptp.tile([128, 1024], BF16, tag="pt",
                                  name=f"pt_{g}_{a}_{gg}_{i}")
                    if not diag:
                        nc.scalar.activation(pt, sts, EXP)
                    elif d == 0:
                        # one full-width exp (cols [512:640) are garbage the
                        # restricted PV never reads); strips masked after
                        nc.scalar.activation(pt, sts, EXP)
                        nc.vector.tensor_mul(
                            pt[:, 0:128], pt[:, 0:128], tri01)
                        nc.vector.tensor_mul(
                            pt[:, 640:768], pt[:, 640:768], tri01)
                    else:
                        # exp only the valid columns; zero the boundary
                        # strip's upper triangle with the 0/1 mask
                        for q2 in range(2):
                            m = 2 * d + q2
                            lo = q2 * 512 + 128 * m
                            hi = (q2 + 1) * 512
                            nc.scalar.activation(
                                pt[:, lo:hi], sts[:, lo:hi], EXP)
                            nc.vector.tensor_mul(
                                pt[:, lo:lo + 128],
                                pt[:, lo:lo + 128], tri01)
                    if pending is not None:
                        emit_pv(*pending)
                        pg, pa, pgg, pi, _ = pending
                        if pgg == 2 * pa + 1:
                            # this pair's PV chain just completed
                            emit_norm_one(pg, pa, pi)
                            if pg == 0 and pa == 3 and pi == 1:
                                inter = emit_y_parts(0, o2s[0], wo_sb, ypool)
                                sched = [ui + 24, ui + 38]
                            if pg == 1 and pa == 3 and pi == 0:
                                # overlap g1's first output-projection half
                                # with the final unit's PV + normalization
                                tail_parts = emit_y_parts(1, o2s[1], wo_sb,
                                                          ypool)
                                tail_parts[0]()
                    pending = (g, a, gg, i, pt)
                emit_pv(*pending)
                emit_norm_one(pending[0], pending[1], pending[3])
                for ch in inter:  # safety: flush any unfired injections
                    ch(next_tag())
                tail_parts[1]()
                return o2s

            def emit_y_parts(g, o2, wo_sb, ypool):
                """Returns closures [part_jh0, part_jh1]; each emits half of
                the output projection so it can interleave with the next
                group's attention stream."""
                ysbs = [ypool.tile([128, 1024], BF16, tag="ysb",
                                   name=f"ysb{g}_{i}") for i in range(2)]

                def part(i, tag=None):
                    for jh in range(2):
                        yps = mmps.tile([128, 512], F32, tag="mm",
                                        name=f"ypsum_{i}_{jh}")
                        for u in range(8):
                            nc.tensor.matmul(
                                yps,
                                lhsT=o2[i][:, u * 128:(u + 1) * 128],
                                rhs=wo_sb[:, u * 1024 + jh * 512:
                                          u * 1024 + (jh + 1) * 512],
                                start=(u == 0), stop=(u == 7),
                            )
                        nc.vector.tensor_add(
                            ysbs[i][:, jh * 512:(jh + 1) * 512], yps,
                            bias_sb[:, jh * 512:(jh + 1) * 512])
                        nc.sync.dma_start(
                            out=out[2 * g + i][:, jh * 512:(jh + 1) * 512],
                            in_=ysbs[i][:, jh * 512:(jh + 1) * 512])

                return [lambda tag=None: part(0), lambda tag=None: part(1)]

            with (
                tc.tile_pool(name="xtp", bufs=1) as xtp,
                tc.tile_pool(name="wp", bufs=1) as wp,
                tc.tile_pool(name="pwork", bufs=1) as pwork,
                tc.tile_pool(name="p2", bufs=1) as p2,
                tc.tile_pool(name="ptp", bufs=6) as ptp,
                tc.tile_pool(name="rp", bufs=2) as rp,
                tc.tile_pool(name="o2p", bufs=2) as o2p,
                tc.tile_pool(name="yp", bufs=2) as ypool,
            ):
                xb_sb = xtp.tile([128, 8, PPC, 128], BF16, tag="xb",
                                 name="xbsb")
                xv = xtb.rearrange("p (kc pr r) -> p kc pr r", kc=8, pr=PPC)
                for kc in range(8):
                    nc.scalar.dma_start(out=xb_sb[:, kc], in_=xv[:, kc])

                w_sbs = []
                for wi, (wparam, dt_) in enumerate(
                        ((wv, BF16), (wq, BF16), (wk, BF16))):
                    w_sb = wp.tile([128, 8, 1024], dt_, tag=f"w{wi}",
                                   name=f"w{wi}")
                    for kc in range(8):
                        nc.sync.dma_start(
                            out=w_sb[:, kc, :],
                            in_=wparam.rearrange(
                                "(c p) j -> p c j", p=128)[:, kc, :])
                    w_sbs.append(w_sb)
                wo_sb = p2.tile([128, 8 * 1024], BF16, tag="wo")
                nc.sync.dma_start(out=wo_sb, in_=wo[:])

                # group 0 runs as early as possible; group 1's projections
                # and pairs 2/3's V path are injected into attention(g0)'s
                # ACT-bound stream at psum-quiet unit indices
                pre_rot = [0]

                def pre_tag():
                    pre_rot[0] += 1
                    return "stA" if pre_rot[0] % 2 == 0 else "stB"

                q0 = qk_chunks(w_sbs[1], xb_sb, qht2, 0, "q")
                q1 = qk_chunks(w_sbs[1], xb_sb, qht2, 1, "q")
                k0 = qk_chunks(w_sbs[2], xb_sb, kht2, 0, "k")
                k1 = qk_chunks(w_sbs[2], xb_sb, kht2, 1, "k")
                for ch in v_chunks(w_sbs[0], xb_sb, pwork, [0, 1]):
                    ch()
                for ch in (q0[0], q1[0], q0[1], q1[1]):
                    ch(pre_tag())
                for ch in (k0[0], k1[0], k0[1], k1[1]):
                    ch(pre_tag())
                for ch in v_chunks(w_sbs[0], xb_sb, pwork, [2, 3]):
                    ch()
                nc.scalar.dma_start(out=bias_sb, in_=bias[:])

                emit_attention_fused(ptp, rp, o2p, ypool, wo_sb)

    nc.finalize()
    return nc


def _host_prep(input_seq_embs, W_Q, W_K, W_V, W_O, b_O):
    X = np.asarray(input_seq_embs, dtype=np.float32)
    WQ = np.asarray(W_Q, dtype=np.float32)
    WK = np.asarray(W_K, dtype=np.float32)
    WV = np.asarray(W_V, dtype=np.float32)
    WO = np.asarray(W_O, dtype=np.float32)
    bO = np.asarray(b_O, dtype=np.float32)

    import ml_dtypes
    bf16 = ml_dtypes.bfloat16

    wq_arr = np.ascontiguousarray(WQ.T).astype(bf16)
    wk_arr = np.ascontiguousarray(WK.T).astype(bf16)
    wv_arr = np.ascontiguousarray(WV.T).astype(bf16)
    # wo[64*parity + j2, 1024*u + jo] = W_O.T[64*(2u+parity) + j2, jo]
    wo_arr = np.ascontiguousarray(
        WO.T.reshape(8, 2, 64, 1024).transpose(1, 2, 0, 3).reshape(
            128, 8192)).astype(bf16)
    bias_arr = np.ascontiguousarray(
        np.broadcast_to(bO, (128, 1024)).astype(np.float32))

    in_maps = []
    for c in range(NCORES):
        # xt[p_, kc, pair, r] = X_pair[r, 128*kc + p_]
        xts = np.empty((128, 8, PPC, 128), dtype=np.float32)
        for p in range(PPC):
            g = PPC * c + p
            bb, hh = g // H, g % H
            Xs = X[bb, 128 * hh:128 * (hh + 1), :]      # (128 r, 1024 cin)
            xts[:, :, p, :] = Xs.T.reshape(8, 128, 128).transpose(1, 0, 2)
        xt_arr = np.ascontiguousarray(xts.reshape(128, 8 * PPC * 128))
        in_maps.append({
            "xtb": xt_arr.astype(bf16),
            "wq": wq_arr, "wk": wk_arr, "wv": wv_arr, "wo": wo_arr,
            "bias": bias_arr,
        })
    return in_maps


_CACHED_NC = None


def get_nc():
    global _CACHED_NC
    if _CACHED_NC is None:
        _CACHED_NC = build_nc()
    return _CACHED_NC


def kernel(**inputs) -> np.ndarray:
    nc = get_nc()
    in_maps = _host_prep(**inputs)
    res = run_bass_kernel_spmd(nc, in_maps, list(range(NCORES)))
    out = np.empty((B, L, D), dtype=np.float32)
    for c in range(NCORES):
        y = np.asarray(res.results[c]["out"],
                       dtype=np.float32)  # (4, 128, 1024)
        for p in range(PPC):
            g = PPC * c + p
            bb, hh = g // H, g % H
            out[bb, 128 * hh:128 * (hh + 1), :] = y[p]
    return out
